# revision 43
# baseline (speedup 1.0000x reference)
"""Multi-head GQA attention (B=2, S=2048, D=2048, H=16, KVH=4) on 8 TRN2
NeuronCores.

Sharding: core i = (b, g) with b = i // 4 (batch), g = i % 4 (sequence
chunk of 512 queries). Each core computes Q for its 512 queries over all
16 heads, K/V for its own 512 sequence positions, AllGathers K/V within
its 4-core batch group, then runs full attention + output projection for
its query chunk. Host concatenates the 8 [512, 2048] chunks.

Layout strategy (no on-chip transposes):
 - host passes x transposed per chunk (xT [D, 512]) so projections
   computed as w.T @ xT yield QT/KT with head-dim on partitions —
   exactly the operand layout attention needs.
 - wq/wk columns permuted per head (even dims first, odd second) so RoPE
   halves are contiguous partition ranges [0:64)/[64:128). Scores are
   permutation-invariant since q and k are permuted identically.
 - scores computed transposed (ST[k, q] = KT.T @ QT) in k-tile PAIRS
   into a [128,1024] PSUM tile (2 banks), exp'd by ONE ScalarE
   activation per pair straight out of PSUM (scale=1/sqrt(HD) folded
   in, no max-subtraction: scores are O(10) so f32 exp is safe). The
   attention phase is ScalarE-exp-bound (~145us), so amortizing the
   ~0.3us/instr ACT overhead across 1024 columns matters.
 - softmax denominator: DVE accumulates the exp'd pair tiles (bf16 2x
   mode), then ONE all-ones-stationary matmul per head replicates the
   denominator across partitions; normalization is reciprocal +
   elementwise multiply, no broadcast. (A per-k-tile ones-matmul
   denominator was 18% of all PE work.)
 - AV matmuls lag the exp pipeline by two pairs so the PE never waits
   on the exp of the tile it just produced.
 - weights are host-pretiled so every DMA is a contiguous block. wo
   streams through the dead wk/wv projection tiles (saves 32KB/
   partition, double-buffered across nt by the WAR deps).
 - engine/sequencer placement is load-bearing: rope swap DMAs on
   ScalarE (on SP their semaphore waits serialize the descriptor chain
   to rope pace), gathered-K/V loads on gpsimd (fire the moment the
   collective completes), everything else on SP. The two AllGathers
   serialize on gpsimd (~20us launch + ~35us transfer each); all 16 Q
   projections run before attention to cover that latency.
"""

import numpy as np
import ml_dtypes

B, S, D = 2, 2048, 2048
H, KVH = 16, 4
HD = D // H            # 128
R = H // KVH           # 4 (GQA repeat)
NCORES = 8
G = 4                  # cores per batch group = seq chunks
SQ = S // G            # 512 queries/keys per core chunk
DKV = KVH * HD         # 512
KS = D // 128          # 16 contraction slices
NKT = S // 128         # 16 key tiles
SCALE = 1.0 / float(np.sqrt(HD))

_CACHE = {}


def _build_nc():
    import concourse.tile as tile
    from concourse import bacc, mybir
    from contextlib import ExitStack

    f32 = mybir.dt.float32
    bf = mybir.dt.bfloat16
    AF = mybir.ActivationFunctionType

    nc = bacc.Bacc("TRN2", target_bir_lowering=False, debug=False, num_devices=NCORES)

    xt_d = nc.dram_tensor("xt", [128, KS * SQ], bf, kind="ExternalInput")
    wq_d = nc.dram_tensor("wq", [H, 128, KS * 128], bf, kind="ExternalInput")
    wk_d = nc.dram_tensor("wk", [128, KS * DKV], bf, kind="ExternalInput")
    wv_d = nc.dram_tensor("wv", [128, KS * DKV], bf, kind="ExternalInput")
    wo_d = nc.dram_tensor("wo", [4, 128, KS * 512], bf, kind="ExternalInput")
    bq_d = nc.dram_tensor("bq", [128, H], f32, kind="ExternalInput")
    bk_d = nc.dram_tensor("bk", [128, KVH], f32, kind="ExternalInput")
    bv_d = nc.dram_tensor("bv", [128, DKV], f32, kind="ExternalInput")
    cos_d = nc.dram_tensor("cosq", [128, SQ], bf, kind="ExternalInput")
    sin_d = nc.dram_tensor("sinq", [128, SQ], bf, kind="ExternalInput")
    ones_d = nc.dram_tensor("ones", [128, 128], bf, kind="ExternalInput")
    perm_d = nc.dram_tensor("perm", [128, 128], bf, kind="ExternalInput")
    out_d = nc.dram_tensor("out", [SQ, D], bf, kind="ExternalOutput")

    # two half-AllGathers, each carrying 2 kv heads' K and V (0.5MB/rank):
    # rows [0:256] = KT of the 2 heads, rows [256:512] = their V halves.
    sendA = nc.dram_tensor("sendA", [8, 128, 256], bf)
    sendB = nc.dram_tensor("sendB", [8, 128, 256], bf)
    fullA = nc.dram_tensor("fullA", [G * 8, 128, 256], bf)
    fullB = nc.dram_tensor("fullB", [G * 8, 128, 256], bf)
    RG = [[0, 1, 2, 3], [4, 5, 6, 7]]

    with tile.TileContext(nc) as tc, ExitStack() as ctx:
        const = ctx.enter_context(tc.tile_pool(name="const", bufs=1))
        big = ctx.enter_context(tc.tile_pool(name="big", bufs=1))
        wqp = ctx.enter_context(tc.tile_pool(name="wqp", bufs=8))
        fp = ctx.enter_context(tc.tile_pool(name="fp", bufs=4))
        rp = ctx.enter_context(tc.tile_pool(name="rp", bufs=4))
        ptp = ctx.enter_context(tc.tile_pool(name="ptp", bufs=6))
        sump = ctx.enter_context(tc.tile_pool(name="sump", bufs=3))
        outp = ctx.enter_context(tc.tile_pool(name="outp", bufs=2))
        recs = ctx.enter_context(tc.tile_pool(name="recs", bufs=2))
        # PSUM budget (8 banks x 2KB): st groups 2x3 banks + av/psw/den/out 2.
        # Projections allocate their [128,512] accumulators as views of the
        # 3-bank st tiles (the pools are static, so proj gets no pool of
        # its own); everything else [128,512]-sized shares pp_av.
        pp_st = ctx.enter_context(tc.tile_pool(name="pp_st", bufs=2, space="PSUM"))
        pp_av = ctx.enter_context(tc.tile_pool(name="pp_av", bufs=2, space="PSUM"))



        # ---------- warmup: HAM busy-window + ACT table preload ----------
        # the PE clock gate defaults to half rate until ~3.4us of sustained
        # activity; dummy matmuls on a memset tile start the busy window at
        # ~7us (during the initial DMA wait) so the real projections run
        # warm almost immediately. The dummy Exp loads the activation table
        # set (~1.3us) off the rope critical path.
        warm = const.tile([128, 512], bf)
        nc.vector.memset(warm[:], 0)
        wps = pp_av.tile([128, 512], f32, tag="av", name="warmps")
        for _ in range(5):
            nc.tensor.matmul(wps[:], warm[:, 0:128], warm[:], start=True, stop=True)
        dume = const.tile([128, 64], bf)
        nc.scalar.activation(dume[:], warm[:, 0:64], AF.Exp)

        # ---------- loads needed by the K/V path, first ----------
        # ONE SP stream in exact consumption order. Transfers drain the
        # shared DMA queues roughly in issue order, so anything issued
        # early that isn't needed early (v2 tried wv/xt tails on other
        # sequencers) steals bandwidth from the K-path slices and stalls
        # the first projections. Total here is ~6MB (~17us of HBM); the
        # K(0,1) sweep consumes its 4MB at about the rate it lands.
        def chunked_load(dst, src_ap, width, n=4, eng=None):
            step = width // n
            for j in range(n):
                e = eng if eng is not None else nc.sync
                e.dma_start(dst[:, j * step:(j + 1) * step], src_ap[:, j * step:(j + 1) * step])

        # xt/wk split into sub-tiles so the first projection chain starts
        # as soon as the first slices land (deps are view-overlap based).
        xt4 = [big.tile([128, 4 * SQ], bf, name=f"xt4_{i}") for i in range(4)]
        wk2 = [big.tile([128, 8 * DKV], bf, name=f"wk2_{i}") for i in range(2)]
        wv_sb = big.tile([128, KS * DKV], bf)
        cos_sb = const.tile([128, SQ], bf)
        sin_sb = const.tile([128, SQ], bf)
        bk_sb = const.tile([128, KVH], f32)
        bv_sb = const.tile([128, DKV], f32)
        perm_sb = const.tile([128, 128], bf)
        # one SP stream in global need order. (Tried alternating SP/ACT to
        # double descriptor issue rate: the ACT-issued descriptors' queue
        # waits occupy the ScalarE FIFO and starve the rope IDENTITYs —
        # +28us. Do NOT put bulk loads on ACT.)
        loads = [
            (wk2[0], (0, 512), wk_d), (xt4[0], (0, 512), xt_d),
            (xt4[0], (512, 1024), xt_d), (wk2[0], (512, 1280), wk_d),
            (xt4[0], (1024, 2048), xt_d), (wk2[0], (1280, 2304), wk_d),
            (wk2[0], (2304, 4096), wk_d),
            (xt4[1], (2048, 3072), xt_d), (xt4[1], (3072, 4096), xt_d),
            (wk2[1], (4096, 5120), wk_d), (xt4[2], (4096, 5120), xt_d),
            (wk2[1], (5120, 6144), wk_d), (xt4[2], (5120, 6144), xt_d),
            (wk2[1], (6144, 7168), wk_d),
            (wk2[1], (7168, 8192), wk_d),
            # small consts: rope (cos/sin/perm) + biases feed the K->send
            # chain at ~19us; behind the whole 6MB they'd arrive too late.
            (cos_sb, None, cos_d), (sin_sb, None, sin_d),
            (bk_sb, None, bk_d), (bv_sb, None, bv_d), (perm_sb, None, perm_d),
        ]
        # late-needed bulk (xt tail for ks 12-15, wv for V-proj) rides
        # gpsimd's three DMA queues, idle until the first sends at ~30us —
        # adds both issue rate and queue-lanes next to SP's nine. Need-
        # ordered, so it never starves the K-path's first slices.
        gp_loads = [
            (xt4[3], (6144, 7168), xt_d), (xt4[3], (7168, 8192), xt_d),
            (wv_sb, (0, 2048), wv_d), (wv_sb, (2048, 4096), wv_d),
            (wv_sb, (4096, 6144), wv_d), (wv_sb, (6144, 8192), wv_d),
        ]
        base = {id(wk2[1]): 4096, id(xt4[1]): 2048, id(xt4[2]): 4096, id(xt4[3]): 6144}

        def do_loads(ls, e):
            for dst, rng, src in ls:
                if rng is None:
                    e.dma_start(dst[:], src.ap())
                else:
                    off = base.get(id(dst), 0)
                    e.dma_start(dst[:, rng[0] - off:rng[1] - off], src.ap()[:, rng[0]:rng[1]])

        do_loads(loads, nc.sync)
        do_loads(gp_loads, nc.gpsimd)

        def xts(ks):
            return xt4[ks // 4][:, (ks % 4) * SQ:(ks % 4 + 1) * SQ]

        # rope, rotate-half form: out = q*[cos;cos] + swap(q)*[-sin;sin].
        # The half-swap is ONE permutation matmul (perm_sb) instead of the
        # old pair of SBUF->SBUF DMAs: those descriptors shared hardware
        # DMA queues with the bulk weight loads, and whenever an AllGather
        # transfer was in flight they crawled, stalling the whole
        # scalar-FIFO -> PSUM-WAR chain behind them (2 stalls, ~17us).
        # rope_start runs on ScalarE right after the projection's last
        # matmul; rope_finish is emitted one head LATER so its P-matmul
        # never waits on the IDENTITY.
        pend = []

        def rope_start(ps_ap, bias_col, dst):
            qf = fp.tile([128, SQ], bf, tag="f")
            nc.scalar.activation(qf[:], ps_ap, AF.Identity, bias=bias_col)
            pend.append((qf, dst))

        def rope_finish():
            if not pend:
                return
            qf, dst = pend.pop(0)
            psw = pp_av.tile([128, SQ], f32, tag="av", name="psw")
            nc.tensor.matmul(psw[:], perm_sb[:], qf[:], start=True, stop=True)
            ta = rp.tile([128, SQ], bf, tag="rt")
            nc.vector.tensor_mul(ta[:], qf[:], cos_sb[:])
            tb = rp.tile([128, SQ], bf, tag="rt")
            nc.vector.tensor_mul(tb[:], psw[:], sin_sb[:])
            nc.vector.tensor_add(dst, ta[:], tb[:])

        # ---------- K/V projection for own chunk, RoPE(K), send ----------
        # order: K heads 0-1 -> V (all) -> AG1 fires early -> K heads 2-3 -> AG2
        kt_own = big.tile([128, KVH * SQ], bf)   # [p=hd, kv*SQ + s]
        v_own = big.tile([128, G * DKV], bf)     # [p=s%128, st*DKV + d]

        def kproj(dt):
            ps = pp_st.tile([128, 3 * SQ], f32, tag="st", name=f"kps{dt}")[:, 0:SQ]
            for ks in range(KS):
                nc.tensor.matmul(
                    ps,
                    wk2[ks // 8][:, (ks % 8) * DKV + dt * 128:(ks % 8) * DKV + (dt + 1) * 128],
                    xts(ks),
                    start=(ks == 0), stop=(ks == KS - 1),
                )
            rope_start(ps, bk_sb[:, dt:dt + 1], kt_own[:, dt * SQ:(dt + 1) * SQ])

        # separate gathered-KV tiles per AG pair (deps are tile-granular);
        # each pair's loads are emitted right after its AG trigger because the
        # collective instruction blocks the gpsimd engine until completion.
        ktfp = [big.tile([128, 2 * S], bf, name=f"ktf{p}") for p in range(2)]
        vfp = [big.tile([128, (G * G) * 256], bf, name=f"vf{p}") for p in range(2)]

        def kv_loads(pair, full_d):
            # on gpsimd: it has a dedicated DMA path and nothing else to do;
            # queued behind the collective they fire the moment it completes.
            # (On SP they'd sit behind the whole Q-proj descriptor chain.)
            ktf_t, vf_t = ktfp[pair], vfp[pair]
            for g in range(G):
                for hh in range(2):
                    for blk in range(2):
                        dst = ktf_t[:, hh * S + g * SQ + blk * 256: hh * S + g * SQ + (blk + 1) * 256]
                        nc.gpsimd.dma_start(dst, full_d.ap()[g * 8 + 2 * hh + blk])
                for st in range(G):
                    dst = vf_t[:, (g * G + st) * 256:(g * G + st) * 256 + 256]
                    nc.gpsimd.dma_start(dst, full_d.ap()[g * 8 + 4 + st])

        def kv_sends(pair, send_d, h0):
            # V halves packed as [128,256] blocks; layout is just bytes,
            # unpacked with matching APs on the receive side.
            for hh in range(2):
                for blk in range(2):
                    src = kt_own[:, (h0 + hh) * SQ + blk * 256:(h0 + hh) * SQ + (blk + 1) * 256]
                    nc.gpsimd.dma_start(send_d.ap()[2 * hh + blk], src)
            for st in range(G):
                src = v_own[:, st * DKV + pair * 256: st * DKV + pair * 256 + 256]
                nc.gpsimd.dma_start(send_d.ap()[4 + st], src)

        def vproj(st):
            ps = pp_st.tile([128, 3 * SQ], f32, tag="st", name=f"vps{st}")[:, 0:DKV]
            for ks in range(KS):
                nc.tensor.matmul(
                    ps,
                    xts(ks)[:, st * 128: st * 128 + 128],
                    wv_sb[:, ks * DKV:(ks + 1) * DKV],
                    start=(ks == 0), stop=(ks == KS - 1),
                )
            nc.vector.tensor_add(v_own[:, st * DKV:(st + 1) * DKV], ps, bv_sb[:])

        kproj(0)
        kproj(1)
        rope_finish()          # K0 (IDENTITY ran during kproj(1))
        vproj(0)
        rope_finish()          # K1
        for st in range(1, G):
            vproj(st)
        kv_sends(0, sendA, 0)
        nc.gpsimd.collective_compute(
            "AllGather", mybir.AluOpType.bypass,
            ins=[sendA.ap()], outs=[fullA.ap()], replica_groups=RG,
        )
        kv_loads(0, fullA)
        kproj(2)
        kproj(3)
        rope_finish()          # K2

        # ---------- remaining consts ----------
        bq_sb = const.tile([128, H], f32)
        ones_sb = const.tile([128, 128], bf)
        nc.sync.dma_start(bq_sb[:], bq_d.ap())
        nc.sync.dma_start(ones_sb[:], ones_d.ap())

        qt4 = [big.tile([128, 4 * SQ], bf, name=f"qt4_{i}") for i in range(4)]  # [p=hd, (h%4)*SQ + q]
        a_sb = big.tile([128, H * SQ], bf)       # [p=hd, h*SQ + q]  (AV^T, normalized)
        # k-tiles per exp group: 3-bank groups amortize the ScalarE ACTIVATE
        # overhead (352 cycles/instr); with pairs the exp chain (8x1147ns)
        # was the attention-phase critical path, above PE's 8.8us/head.
        GROUPS = (3, 3, 3, 3, 2, 2)
        GBASE = (0, 3, 6, 9, 12, 14)
        # deferred tails of the previous head, drained inside the next
        # head's early groups so the PE's score stream (which feeds the
        # exp pipeline) is never delayed by the previous head's epilogue:
        # avpend = its last two AV groups, dpend = (folds, den+normalize).
        avpend = []
        dpend = []

        def qproj_head(ht):
            wq_t = wqp.tile([128, KS * 128], bf, tag="wq")
            for j in range(4):
                nc.sync.dma_start(wq_t[:, j * 512:(j + 1) * 512], wq_d.ap()[ht][:, j * 512:(j + 1) * 512])
            ps = pp_st.tile([128, 3 * SQ], f32, tag="st", name=f"qps{ht}")[:, 0:SQ]
            for ks in range(KS):
                nc.tensor.matmul(
                    ps,
                    wq_t[:, ks * 128:(ks + 1) * 128],
                    xts(ks),
                    start=(ks == 0), stop=(ks == KS - 1),
                )
            rope_start(ps, bq_sb[:, ht:ht + 1], qt4[ht // 4][:, (ht % 4) * SQ:(ht % 4 + 1) * SQ])

        def attn_head(h):
            # scores + exp in k-tile GROUPS (3,3,3,3,2,2 over 3-bank PSUM
            # tiles): one ACTIVATE per group; softmax denominator via DVE
            # group-sums + ONE matmul per head, deferred into the NEXT
            # head's pipeline (emitted after its group-0 scores) so the PE
            # never waits on the last exp + fold chain. The den result goes
            # into the spare third bank of the head's final 2-wide group.
            kv = h // R
            ktf_t, vf_t = ktfp[kv // 2], vfp[kv // 2]
            kvh = kv % 2
            av = pp_av.tile([128, SQ], f32, tag="av", name=f"av{h}")
            ptsum = sump.tile([128, 3 * SQ], bf, tag="ptsum")
            pts = [None] * 6
            qsl = qt4[h // 4][:, (h % 4) * SQ:(h % 4 + 1) * SQ]

            def av_group(g):
                for j in range(GROUPS[g]):
                    kt = GBASE[g] + j
                    nc.tensor.matmul(
                        av[:],
                        vf_t[:, kt * 256 + kvh * 128: kt * 256 + (kvh + 1) * 128],
                        pts[g][:, j * SQ:(j + 1) * SQ],
                        start=(kt == 0), stop=(kt == NKT - 1),
                    )

            # AV runs two groups behind scores/exp so the PE never waits on
            # the exp of the group it just produced.
            for g in range(6):
                gsz = GROUPS[g]
                st_ps = pp_st.tile([128, 3 * SQ], f32, tag="st")
                for j in range(gsz):
                    kt = GBASE[g] + j
                    nc.tensor.matmul(
                        st_ps[:, j * SQ:(j + 1) * SQ],
                        ktf_t[:, kvh * S + kt * 128: kvh * S + (kt + 1) * 128],
                        qsl,
                        start=True, stop=True,
                    )
                if g == 0:
                    while avpend:
                        avpend.pop(0)()          # head h-1's av4/av5
                    if dpend:
                        dpend[0][0]()            # head h-1's den folds (DVE)
                elif g == 1 and dpend:
                    dpend.pop(0)[1]()            # head h-1's den/normalize
                w = gsz * SQ
                pt = ptp.tile([128, 3 * SQ], bf, tag="pt")
                nc.scalar.activation(pt[:, 0:w], st_ps[:, 0:w], AF.Exp, scale=SCALE)
                pts[g] = pt
                if g == 1:
                    nc.vector.tensor_add(ptsum[:], pts[0][:], pts[1][:])
                elif g > 1:
                    nc.vector.tensor_add(ptsum[:, 0:w], ptsum[:, 0:w], pt[:, 0:w])
                if g >= 2:
                    av_group(g - 2)
            avpend.append(lambda: av_group(4))
            avpend.append(lambda: av_group(5))

            # den tile allocated EAGERLY (keeps the av/den alternation on
            # pp_av's two bufs intact) but its instructions are deferred.
            den_t = pp_av.tile([128, SQ], f32, tag="av", name=f"den{h}")
            box = {}

            def den_folds(ptsum=ptsum, box=box):
                ps512 = sump.tile([128, SQ], bf, tag="ps512")
                nc.vector.tensor_add(ps512[:], ptsum[:, 0:SQ], ptsum[:, SQ:2 * SQ])
                nc.vector.tensor_add(ps512[:], ps512[:], ptsum[:, 2 * SQ:3 * SQ])
                box["ps512"] = ps512

            def den_norm(h=h, av=av, den_t=den_t, box=box):
                nc.tensor.matmul(den_t[:], ones_sb[:], box["ps512"][:], start=True, stop=True)
                recb = recs.tile([128, SQ], f32, tag="recb")
                nc.vector.reciprocal_approx_fast(recb[:], den_t[:])
                nc.vector.tensor_mul(a_sb[:, h * SQ:(h + 1) * SQ], av[:], recb[:])
            dpend.append((den_folds, den_norm))

        # ---------- schedule ----------
        # Q projections before attention: the PE work covers the first
        # AllGather's fire-to-complete latency, so attention heads 0-7
        # start right as the gathered K/V lands. The second AllGather's
        # sends need kt_own dt=2,3, so they're emitted after qproj(0) has
        # flushed the K3 rope.
        qproj_head(0)
        rope_finish()          # K3
        kv_sends(1, sendB, 2)
        nc.gpsimd.collective_compute(
            "AllGather", mybir.AluOpType.bypass,
            ins=[sendB.ap()], outs=[fullB.ap()], replica_groups=RG,
        )
        kv_loads(1, fullB)
        for i in range(1, H):
            qproj_head(i)
            rope_finish()      # Q[i-1]
        rope_finish()          # Q15
        for h in range(H):
            attn_head(h)

        # ---------- output projection ----------
        # wo streams through dead projection-phase weight tiles: wv_sb
        # (nt=0), wk2 (nt=1), and the wq pool's 8 bufs (nt=2,3) — all idle
        # after Q proj. The loads go on gpsimd: its DMA queues (166-168)
        # are disjoint from the rope-swap/weight-load queues, so a gated or
        # queued wo descriptor can never block the attention pipeline (the
        # old SP+time-gate scheme stalled PE 12.5us mid-attention). gpsimd
        # reaches these descriptors at ~155us, right after the second
        # AllGather's K/V loads; transfers finish long before out-proj.
        wo_sp = [wqp.tile([128, 4 * 512], bf, tag="wq", name=f"wosp{i}") for i in range(8)]

        def wo_slice(nt, ct):
            if nt == 0:
                return wv_sb[:, ct * 512:(ct + 1) * 512]
            if nt == 1:
                t = wk2[ct // 8]
                return t[:, (ct % 8) * 512:((ct % 8) + 1) * 512]
            t = wo_sp[(nt - 2) * 4 + ct // 4]
            return t[:, (ct % 4) * 512:((ct % 4) + 1) * 512]

        for nt in range(4):
            for ct in range(KS):
                nc.gpsimd.dma_start(wo_slice(nt, ct), wo_d.ap()[nt][:, ct * 512:(ct + 1) * 512])
        # head 15's epilogue: av tail + den folds now; the den matmul and
        # normalize are emitted INSIDE the first out-proj group (which runs
        # on a pp_st tile, so the pp_av rotation stays clobber-safe), after
        # ct=12 — by ct=15 the normalized a_sb[15] is ready.
        while avpend:
            avpend.pop(0)()
        dpend[0][0]()
        for nt in range(4):
            for qt in range(4):
                if nt == 0 and qt == 0:
                    pst = pp_st.tile([128, 3 * SQ], f32, tag="st", name="ops00")
                else:
                    pst = pp_av.tile([128, 512], f32, tag="av", name=f"ops{nt}_{qt}")
                for ct in range(KS):
                    nc.tensor.matmul(
                        pst[:, 0:512],
                        a_sb[:, ct * SQ + qt * 128: ct * SQ + qt * 128 + 128],
                        wo_slice(nt, ct),
                        start=(ct == 0), stop=(ct == KS - 1),
                    )
                    if nt == 0 and qt == 0 and ct == 12:
                        dpend.pop(0)[1]()   # den(15) + normalize
                ot = outp.tile([128, 512], bf, tag="ot")
                if nt == 3 and qt == 3:
                    # split the last tile so the final store overlaps the copy
                    for c0 in (0, 256):
                        nc.scalar.activation(ot[:, c0:c0 + 256], pst[:, c0:c0 + 256], AF.Copy)
                        nc.sync.dma_start(
                            out_d.ap()[qt * 128:(qt + 1) * 128, nt * 512 + c0:nt * 512 + c0 + 256],
                            ot[:, c0:c0 + 256],
                        )
                else:
                    nc.scalar.activation(ot[:], pst[:, 0:512], AF.Copy)
                    nc.sync.dma_start(out_d.ap()[qt * 128:(qt + 1) * 128, nt * 512:(nt + 1) * 512], ot[:])

    nc.compile()
    return nc


def get_nc():
    if "nc" not in _CACHE:
        _CACHE["nc"] = _build_nc()
    return _CACHE["nc"]


def make_in_maps(x, wq, bq, wk, bk, wv, bv, wo):
    bf16 = ml_dtypes.bfloat16
    perm = np.concatenate([np.arange(0, HD, 2), np.arange(1, HD, 2)])
    qcols = np.concatenate([h * HD + perm for h in range(H)])
    kcols = np.concatenate([h * HD + perm for h in range(KVH)])
    wq_p = wq[:, qcols]
    bq_p = np.ascontiguousarray(bq[qcols].reshape(H, HD).T).astype(np.float32)
    wk_p = wk[:, kcols]
    bk_p = np.ascontiguousarray(bk[kcols].reshape(KVH, HD).T).astype(np.float32)
    # pretile so every DMA is contiguous: wq [ht][p][ks][c], wk/wv [p][ks][c],
    # wo [nt][p][ct][c]
    wq_t = np.ascontiguousarray(
        wq_p.reshape(KS, 128, H, 128).transpose(2, 1, 0, 3).reshape(H, 128, KS * 128)
    ).astype(bf16)
    wk_t = np.ascontiguousarray(
        wk_p.reshape(KS, 128, DKV).transpose(1, 0, 2).reshape(128, KS * DKV)
    ).astype(bf16)
    wv_t = np.ascontiguousarray(
        wv.reshape(KS, 128, DKV).transpose(1, 0, 2).reshape(128, KS * DKV)
    ).astype(bf16)
    wo_t = np.ascontiguousarray(
        wo.reshape(KS, 128, 4, 512).transpose(2, 1, 0, 3).reshape(4, 128, KS * 512)
    ).astype(bf16)
    bv_rep = np.tile(bv.astype(np.float32), (128, 1))
    theta = (10000.0 ** (-np.arange(64, dtype=np.float64) / 64.0))
    ang = np.outer(np.arange(S, dtype=np.float64), theta)  # [S, 64]
    c = np.cos(ang).T.astype(np.float32)  # [64, S]
    s = np.sin(ang).T.astype(np.float32)
    cosT = np.concatenate([c, c], axis=0)      # [128, S]
    sinT = np.concatenate([-s, s], axis=0)     # [128, S]
    ones = np.ones((128, 128), dtype=bf16)
    # psw = perm.T @ qf must be the half-swap: psw[i] = qf[(i+64)%128]
    perm = np.roll(np.eye(128, dtype=np.float32), 64, axis=0).astype(bf16)

    in_maps = []
    for b in range(B):
        for g in range(G):
            sl = slice(g * SQ, (g + 1) * SQ)
            xt_c = np.ascontiguousarray(
                x[b, sl, :].T.reshape(KS, 128, SQ).transpose(1, 0, 2).reshape(128, KS * SQ)
            ).astype(bf16)
            in_maps.append({
                "xt": xt_c,
                "wq": wq_t, "wk": wk_t, "wv": wv_t, "wo": wo_t,
                "bq": bq_p, "bk": bk_p, "bv": bv_rep,
                "cosq": np.ascontiguousarray(cosT[:, sl]).astype(bf16),
                "sinq": np.ascontiguousarray(sinT[:, sl]).astype(bf16),
                "ones": ones,
                "perm": perm,
            })
    return in_maps


def assemble(results):
    out = np.empty((B, S, D), np.float32)
    for b in range(B):
        for g in range(G):
            out[b, g * SQ:(g + 1) * SQ, :] = results[b * G + g]["out"]
    return out


def kernel(x, wq, bq, wk, bk, wv, bv, wo):
    from concourse.bass_utils import run_bass_kernel_spmd

    x, wq, bq, wk, bk, wv, bv, wo = (
        np.asarray(t, dtype=np.float32) for t in (x, wq, bq, wk, bk, wv, bv, wo)
    )
    nc = get_nc()
    in_maps = make_in_maps(x, wq, bq, wk, bk, wv, bv, wo)
    # run twice and return the second result: the first execution after a
    # NEFF load has occasionally produced stale collective output.
    run_bass_kernel_spmd(nc, in_maps, core_ids=list(range(NCORES)))
    res = run_bass_kernel_spmd(nc, in_maps, core_ids=list(range(NCORES)))
    return assemble(res.results)



# revision 44
# speedup vs baseline: 1.0141x; 1.0141x over previous
"""Multi-head GQA attention (B=2, S=2048, D=2048, H=16, KVH=4) on 8 TRN2
NeuronCores.

Sharding: core i = (b, g) with b = i // 4 (batch), g = i % 4 (sequence
chunk of 512 queries). Each core computes Q for its 512 queries over all
16 heads, K/V for its own 512 sequence positions, AllGathers K/V within
its 4-core batch group, then runs full attention + output projection for
its query chunk. Host concatenates the 8 [512, 2048] chunks.

Layout strategy (no on-chip transposes):
 - host passes x transposed per chunk (xT [D, 512]) so projections
   computed as w.T @ xT yield QT/KT with head-dim on partitions —
   exactly the operand layout attention needs.
 - wq/wk columns permuted per head (even dims first, odd second) so RoPE
   halves are contiguous partition ranges [0:64)/[64:128). Scores are
   permutation-invariant since q and k are permuted identically.
 - scores computed transposed (ST[k, q] = KT.T @ QT) in k-tile PAIRS
   into a [128,1024] PSUM tile (2 banks), exp'd by ONE ScalarE
   activation per pair straight out of PSUM (scale=1/sqrt(HD) folded
   in, no max-subtraction: scores are O(10) so f32 exp is safe). The
   attention phase is ScalarE-exp-bound (~145us), so amortizing the
   ~0.3us/instr ACT overhead across 1024 columns matters.
 - softmax denominator: DVE accumulates the exp'd pair tiles (bf16 2x
   mode), then ONE all-ones-stationary matmul per head replicates the
   denominator across partitions; normalization is reciprocal +
   elementwise multiply, no broadcast. (A per-k-tile ones-matmul
   denominator was 18% of all PE work.)
 - AV matmuls lag the exp pipeline by two pairs so the PE never waits
   on the exp of the tile it just produced.
 - weights are host-pretiled so every DMA is a contiguous block. wo
   streams through the dead wk/wv projection tiles (saves 32KB/
   partition, double-buffered across nt by the WAR deps).
 - engine/sequencer placement is load-bearing: rope swap DMAs on
   ScalarE (on SP their semaphore waits serialize the descriptor chain
   to rope pace), gathered-K/V loads on gpsimd (fire the moment the
   collective completes), everything else on SP. The two AllGathers
   serialize on gpsimd (~20us launch + ~35us transfer each); all 16 Q
   projections run before attention to cover that latency.
"""

import numpy as np
import ml_dtypes

B, S, D = 2, 2048, 2048
H, KVH = 16, 4
HD = D // H            # 128
R = H // KVH           # 4 (GQA repeat)
NCORES = 8
G = 4                  # cores per batch group = seq chunks
SQ = S // G            # 512 queries/keys per core chunk
DKV = KVH * HD         # 512
KS = D // 128          # 16 contraction slices
NKT = S // 128         # 16 key tiles
SCALE = 1.0 / float(np.sqrt(HD))

_CACHE = {}


def _build_nc():
    import concourse.tile as tile
    from concourse import bacc, mybir
    from contextlib import ExitStack

    f32 = mybir.dt.float32
    bf = mybir.dt.bfloat16
    AF = mybir.ActivationFunctionType

    nc = bacc.Bacc("TRN2", target_bir_lowering=False, debug=False, num_devices=NCORES)

    xt_d = nc.dram_tensor("xt", [128, KS * SQ], bf, kind="ExternalInput")
    wq_d = nc.dram_tensor("wq", [H, 128, KS * 128], bf, kind="ExternalInput")
    wk_d = nc.dram_tensor("wk", [128, KS * DKV], bf, kind="ExternalInput")
    wv_d = nc.dram_tensor("wv", [128, KS * DKV], bf, kind="ExternalInput")
    wo_d = nc.dram_tensor("wo", [4, 128, KS * 512], bf, kind="ExternalInput")
    bq_d = nc.dram_tensor("bq", [128, H], f32, kind="ExternalInput")
    bk_d = nc.dram_tensor("bk", [128, KVH], f32, kind="ExternalInput")
    bv_d = nc.dram_tensor("bv", [128, DKV], f32, kind="ExternalInput")
    cos_d = nc.dram_tensor("cosq", [128, SQ], bf, kind="ExternalInput")
    sin_d = nc.dram_tensor("sinq", [128, SQ], bf, kind="ExternalInput")
    ones_d = nc.dram_tensor("ones", [128, 128], bf, kind="ExternalInput")
    perm_d = nc.dram_tensor("perm", [128, 128], bf, kind="ExternalInput")
    out_d = nc.dram_tensor("out", [SQ, D], bf, kind="ExternalOutput")

    # two half-AllGathers, each carrying 2 kv heads' K and V (0.5MB/rank):
    # rows [0:256] = KT of the 2 heads, rows [256:512] = their V halves.
    sendA = nc.dram_tensor("sendA", [8, 128, 256], bf)
    sendB = nc.dram_tensor("sendB", [8, 128, 256], bf)
    fullA = nc.dram_tensor("fullA", [G * 8, 128, 256], bf)
    fullB = nc.dram_tensor("fullB", [G * 8, 128, 256], bf)
    RG = [[0, 1, 2, 3], [4, 5, 6, 7]]

    with tile.TileContext(nc) as tc, ExitStack() as ctx:
        const = ctx.enter_context(tc.tile_pool(name="const", bufs=1))
        big = ctx.enter_context(tc.tile_pool(name="big", bufs=1))
        wqp = ctx.enter_context(tc.tile_pool(name="wqp", bufs=8))
        fp = ctx.enter_context(tc.tile_pool(name="fp", bufs=4))
        rp = ctx.enter_context(tc.tile_pool(name="rp", bufs=4))
        ptp = ctx.enter_context(tc.tile_pool(name="ptp", bufs=6))
        sump = ctx.enter_context(tc.tile_pool(name="sump", bufs=3))
        outp = ctx.enter_context(tc.tile_pool(name="outp", bufs=2))
        recs = ctx.enter_context(tc.tile_pool(name="recs", bufs=2))
        # PSUM budget (8 banks x 2KB): st groups 2x3 banks + av/psw/den/out 2.
        # Projections allocate their [128,512] accumulators as views of the
        # 3-bank st tiles (the pools are static, so proj gets no pool of
        # its own); everything else [128,512]-sized shares pp_av.
        pp_st = ctx.enter_context(tc.tile_pool(name="pp_st", bufs=2, space="PSUM"))
        pp_av = ctx.enter_context(tc.tile_pool(name="pp_av", bufs=2, space="PSUM"))



        # ---------- warmup: HAM busy-window + ACT table preload ----------
        # the PE clock gate defaults to half rate until ~3.4us of sustained
        # activity; dummy matmuls on a memset tile start the busy window at
        # ~7us (during the initial DMA wait) so the real projections run
        # warm almost immediately. The dummy Exp loads the activation table
        # set (~1.3us) off the rope critical path.
        warm = const.tile([128, 512], bf)
        nc.vector.memset(warm[:], 0)
        wps = pp_av.tile([128, 512], f32, tag="av", name="warmps")
        for _ in range(5):
            nc.tensor.matmul(wps[:], warm[:, 0:128], warm[:], start=True, stop=True)
        dume = const.tile([128, 64], bf)
        nc.scalar.activation(dume[:], warm[:, 0:64], AF.Exp)

        # ---------- loads needed by the K/V path, first ----------
        # ONE SP stream in exact consumption order. Transfers drain the
        # shared DMA queues roughly in issue order, so anything issued
        # early that isn't needed early (v2 tried wv/xt tails on other
        # sequencers) steals bandwidth from the K-path slices and stalls
        # the first projections. Total here is ~6MB (~17us of HBM); the
        # K(0,1) sweep consumes its 4MB at about the rate it lands.
        def chunked_load(dst, src_ap, width, n=4, eng=None):
            step = width // n
            for j in range(n):
                e = eng if eng is not None else nc.sync
                e.dma_start(dst[:, j * step:(j + 1) * step], src_ap[:, j * step:(j + 1) * step])

        # xt/wk split into sub-tiles so the first projection chain starts
        # as soon as the first slices land (deps are view-overlap based).
        xt4 = [big.tile([128, 4 * SQ], bf, name=f"xt4_{i}") for i in range(4)]
        wk2 = [big.tile([128, 8 * DKV], bf, name=f"wk2_{i}") for i in range(2)]
        wv_sb = big.tile([128, KS * DKV], bf)
        cos_sb = const.tile([128, SQ], bf)
        sin_sb = const.tile([128, SQ], bf)
        bk_sb = const.tile([128, KVH], f32)
        bv_sb = const.tile([128, DKV], f32)
        perm_sb = const.tile([128, 128], bf)
        # one SP stream in global need order. (Tried alternating SP/ACT to
        # double descriptor issue rate: the ACT-issued descriptors' queue
        # waits occupy the ScalarE FIFO and starve the rope IDENTITYs —
        # +28us. Do NOT put bulk loads on ACT.)
        loads = [
            (wk2[0], (0, 512), wk_d), (xt4[0], (0, 512), xt_d),
            (xt4[0], (512, 1024), xt_d), (wk2[0], (512, 1280), wk_d),
            (xt4[0], (1024, 2048), xt_d), (wk2[0], (1280, 2304), wk_d),
            (wk2[0], (2304, 4096), wk_d),
            (xt4[1], (2048, 3072), xt_d), (xt4[1], (3072, 4096), xt_d),
            (wk2[1], (4096, 5120), wk_d), (xt4[2], (4096, 5120), xt_d),
            (wk2[1], (5120, 6144), wk_d), (xt4[2], (5120, 6144), xt_d),
            (wk2[1], (6144, 7168), wk_d), (xt4[3], (6144, 7168), xt_d),
            (wk2[1], (7168, 8192), wk_d), (xt4[3], (7168, 8192), xt_d),
            # small consts: rope (cos/sin/perm) + biases feed the K->send
            # chain at ~19us; behind the whole 6MB they'd arrive too late.
            (cos_sb, None, cos_d), (sin_sb, None, sin_d),
            (bk_sb, None, bk_d), (bv_sb, None, bv_d), (perm_sb, None, perm_d),
            (wv_sb, (0, 2048), wv_d), (wv_sb, (2048, 4096), wv_d),
            (wv_sb, (4096, 6144), wv_d), (wv_sb, (6144, 8192), wv_d),
        ]
        # (Tried offloading the xt tail + wv onto gpsimd's three idle DMA
        # queues for extra issue rate: consistently ~3us slower. Keep ONE
        # need-ordered SP stream.)
        base = {id(wk2[1]): 4096, id(xt4[1]): 2048, id(xt4[2]): 4096, id(xt4[3]): 6144}
        for dst, rng, src in loads:
            if rng is None:
                nc.sync.dma_start(dst[:], src.ap())
            else:
                off = base.get(id(dst), 0)
                nc.sync.dma_start(dst[:, rng[0] - off:rng[1] - off], src.ap()[:, rng[0]:rng[1]])

        def xts(ks):
            return xt4[ks // 4][:, (ks % 4) * SQ:(ks % 4 + 1) * SQ]

        # rope, rotate-half form: out = q*[cos;cos] + swap(q)*[-sin;sin].
        # The half-swap is ONE permutation matmul (perm_sb) instead of the
        # old pair of SBUF->SBUF DMAs: those descriptors shared hardware
        # DMA queues with the bulk weight loads, and whenever an AllGather
        # transfer was in flight they crawled, stalling the whole
        # scalar-FIFO -> PSUM-WAR chain behind them (2 stalls, ~17us).
        # rope_start runs on ScalarE right after the projection's last
        # matmul; rope_finish is emitted one head LATER so its P-matmul
        # never waits on the IDENTITY.
        pend = []

        def rope_start(ps_ap, bias_col, dst):
            qf = fp.tile([128, SQ], bf, tag="f")
            nc.scalar.activation(qf[:], ps_ap, AF.Identity, bias=bias_col)
            pend.append((qf, dst))

        def rope_finish():
            if not pend:
                return
            qf, dst = pend.pop(0)
            psw = pp_av.tile([128, SQ], f32, tag="av", name="psw")
            nc.tensor.matmul(psw[:], perm_sb[:], qf[:], start=True, stop=True)
            ta = rp.tile([128, SQ], bf, tag="rt")
            nc.vector.tensor_mul(ta[:], qf[:], cos_sb[:])
            tb = rp.tile([128, SQ], bf, tag="rt")
            nc.vector.tensor_mul(tb[:], psw[:], sin_sb[:])
            nc.vector.tensor_add(dst, ta[:], tb[:])

        # ---------- K/V projection for own chunk, RoPE(K), send ----------
        # order: K heads 0-1 -> V (all) -> AG1 fires early -> K heads 2-3 -> AG2
        kt_own = big.tile([128, KVH * SQ], bf)   # [p=hd, kv*SQ + s]
        v_own = big.tile([128, G * DKV], bf)     # [p=s%128, st*DKV + d]

        def kproj(dt):
            ps = pp_st.tile([128, 3 * SQ], f32, tag="st", name=f"kps{dt}")[:, 0:SQ]
            for ks in range(KS):
                nc.tensor.matmul(
                    ps,
                    wk2[ks // 8][:, (ks % 8) * DKV + dt * 128:(ks % 8) * DKV + (dt + 1) * 128],
                    xts(ks),
                    start=(ks == 0), stop=(ks == KS - 1),
                )
            rope_start(ps, bk_sb[:, dt:dt + 1], kt_own[:, dt * SQ:(dt + 1) * SQ])

        # separate gathered-KV tiles per AG pair (deps are tile-granular);
        # each pair's loads are emitted right after its AG trigger because the
        # collective instruction blocks the gpsimd engine until completion.
        ktfp = [big.tile([128, 2 * S], bf, name=f"ktf{p}") for p in range(2)]
        vfp = [big.tile([128, (G * G) * 256], bf, name=f"vf{p}") for p in range(2)]

        def kv_loads(pair, full_d):
            # on gpsimd: it has a dedicated DMA path and nothing else to do;
            # queued behind the collective they fire the moment it completes.
            # (On SP they'd sit behind the whole Q-proj descriptor chain.)
            ktf_t, vf_t = ktfp[pair], vfp[pair]
            for g in range(G):
                for hh in range(2):
                    for blk in range(2):
                        dst = ktf_t[:, hh * S + g * SQ + blk * 256: hh * S + g * SQ + (blk + 1) * 256]
                        nc.gpsimd.dma_start(dst, full_d.ap()[g * 8 + 2 * hh + blk])
                for st in range(G):
                    dst = vf_t[:, (g * G + st) * 256:(g * G + st) * 256 + 256]
                    nc.gpsimd.dma_start(dst, full_d.ap()[g * 8 + 4 + st])

        def kv_sends(pair, send_d, h0):
            # V halves packed as [128,256] blocks; layout is just bytes,
            # unpacked with matching APs on the receive side.
            for hh in range(2):
                for blk in range(2):
                    src = kt_own[:, (h0 + hh) * SQ + blk * 256:(h0 + hh) * SQ + (blk + 1) * 256]
                    nc.gpsimd.dma_start(send_d.ap()[2 * hh + blk], src)
            for st in range(G):
                src = v_own[:, st * DKV + pair * 256: st * DKV + pair * 256 + 256]
                nc.gpsimd.dma_start(send_d.ap()[4 + st], src)

        def vproj(st):
            ps = pp_st.tile([128, 3 * SQ], f32, tag="st", name=f"vps{st}")[:, 0:DKV]
            for ks in range(KS):
                nc.tensor.matmul(
                    ps,
                    xts(ks)[:, st * 128: st * 128 + 128],
                    wv_sb[:, ks * DKV:(ks + 1) * DKV],
                    start=(ks == 0), stop=(ks == KS - 1),
                )
            nc.vector.tensor_add(v_own[:, st * DKV:(st + 1) * DKV], ps, bv_sb[:])

        kproj(0)
        kproj(1)
        rope_finish()          # K0 (IDENTITY ran during kproj(1))
        vproj(0)
        rope_finish()          # K1
        for st in range(1, G):
            vproj(st)
        kv_sends(0, sendA, 0)
        nc.gpsimd.collective_compute(
            "AllGather", mybir.AluOpType.bypass,
            ins=[sendA.ap()], outs=[fullA.ap()], replica_groups=RG,
        )
        kv_loads(0, fullA)
        kproj(2)
        kproj(3)
        rope_finish()          # K2

        # ---------- remaining consts ----------
        bq_sb = const.tile([128, H], f32)
        ones_sb = const.tile([128, 128], bf)
        nc.sync.dma_start(bq_sb[:], bq_d.ap())
        nc.sync.dma_start(ones_sb[:], ones_d.ap())

        qt4 = [big.tile([128, 4 * SQ], bf, name=f"qt4_{i}") for i in range(4)]  # [p=hd, (h%4)*SQ + q]
        a_sb = big.tile([128, H * SQ], bf)       # [p=hd, h*SQ + q]  (AV^T, normalized)
        # k-tiles per exp group: 3-bank groups amortize the ScalarE ACTIVATE
        # overhead (352 cycles/instr); with pairs the exp chain (8x1147ns)
        # was the attention-phase critical path, above PE's 8.8us/head.
        GROUPS = (3, 3, 3, 3, 2, 2)
        GBASE = (0, 3, 6, 9, 12, 14)
        # deferred tails of the previous head, drained inside the next
        # head's early groups so the PE's score stream (which feeds the
        # exp pipeline) is never delayed by the previous head's epilogue:
        # avpend = its last two AV groups, dpend = (folds, den+normalize).
        avpend = []
        dpend = []

        def qproj_head(ht):
            wq_t = wqp.tile([128, KS * 128], bf, tag="wq")
            for j in range(4):
                nc.sync.dma_start(wq_t[:, j * 512:(j + 1) * 512], wq_d.ap()[ht][:, j * 512:(j + 1) * 512])
            ps = pp_st.tile([128, 3 * SQ], f32, tag="st", name=f"qps{ht}")[:, 0:SQ]
            for ks in range(KS):
                nc.tensor.matmul(
                    ps,
                    wq_t[:, ks * 128:(ks + 1) * 128],
                    xts(ks),
                    start=(ks == 0), stop=(ks == KS - 1),
                )
            rope_start(ps, bq_sb[:, ht:ht + 1], qt4[ht // 4][:, (ht % 4) * SQ:(ht % 4 + 1) * SQ])

        def attn_head(h):
            # scores + exp in k-tile GROUPS (3,3,3,3,2,2 over 3-bank PSUM
            # tiles): one ACTIVATE per group; softmax denominator via DVE
            # group-sums + ONE matmul per head, deferred into the NEXT
            # head's pipeline (emitted after its group-0 scores) so the PE
            # never waits on the last exp + fold chain. The den result goes
            # into the spare third bank of the head's final 2-wide group.
            kv = h // R
            ktf_t, vf_t = ktfp[kv // 2], vfp[kv // 2]
            kvh = kv % 2
            av = pp_av.tile([128, SQ], f32, tag="av", name=f"av{h}")
            ptsum = sump.tile([128, 3 * SQ], bf, tag="ptsum")
            pts = [None] * 6
            qsl = qt4[h // 4][:, (h % 4) * SQ:(h % 4 + 1) * SQ]

            def av_group(g):
                for j in range(GROUPS[g]):
                    kt = GBASE[g] + j
                    nc.tensor.matmul(
                        av[:],
                        vf_t[:, kt * 256 + kvh * 128: kt * 256 + (kvh + 1) * 128],
                        pts[g][:, j * SQ:(j + 1) * SQ],
                        start=(kt == 0), stop=(kt == NKT - 1),
                    )

            # AV runs two groups behind scores/exp so the PE never waits on
            # the exp of the group it just produced.
            for g in range(6):
                gsz = GROUPS[g]
                st_ps = pp_st.tile([128, 3 * SQ], f32, tag="st")
                for j in range(gsz):
                    kt = GBASE[g] + j
                    nc.tensor.matmul(
                        st_ps[:, j * SQ:(j + 1) * SQ],
                        ktf_t[:, kvh * S + kt * 128: kvh * S + (kt + 1) * 128],
                        qsl,
                        start=True, stop=True,
                    )
                if g == 0:
                    while avpend:
                        avpend.pop(0)()          # head h-1's av4/av5
                    if dpend:
                        dpend[0][0]()            # head h-1's den folds (DVE)
                elif g == 1 and dpend:
                    dpend.pop(0)[1]()            # head h-1's den/normalize
                w = gsz * SQ
                pt = ptp.tile([128, 3 * SQ], bf, tag="pt")
                nc.scalar.activation(pt[:, 0:w], st_ps[:, 0:w], AF.Exp, scale=SCALE)
                pts[g] = pt
                if g == 1:
                    nc.vector.tensor_add(ptsum[:], pts[0][:], pts[1][:])
                elif g > 1:
                    nc.vector.tensor_add(ptsum[:, 0:w], ptsum[:, 0:w], pt[:, 0:w])
                if g >= 2:
                    av_group(g - 2)
            avpend.append(lambda: av_group(4))
            avpend.append(lambda: av_group(5))

            # den tile allocated EAGERLY (keeps the av/den alternation on
            # pp_av's two bufs intact) but its instructions are deferred.
            den_t = pp_av.tile([128, SQ], f32, tag="av", name=f"den{h}")
            box = {}

            def den_folds(ptsum=ptsum, box=box):
                ps512 = sump.tile([128, SQ], bf, tag="ps512")
                nc.vector.tensor_add(ps512[:], ptsum[:, 0:SQ], ptsum[:, SQ:2 * SQ])
                nc.vector.tensor_add(ps512[:], ps512[:], ptsum[:, 2 * SQ:3 * SQ])
                box["ps512"] = ps512

            def den_norm(h=h, av=av, den_t=den_t, box=box):
                nc.tensor.matmul(den_t[:], ones_sb[:], box["ps512"][:], start=True, stop=True)
                recb = recs.tile([128, SQ], f32, tag="recb")
                nc.vector.reciprocal_approx_fast(recb[:], den_t[:])
                nc.vector.tensor_mul(a_sb[:, h * SQ:(h + 1) * SQ], av[:], recb[:])
            dpend.append((den_folds, den_norm))

        # ---------- schedule ----------
        # Q projections before attention: the PE work covers the first
        # AllGather's fire-to-complete latency, so attention heads 0-7
        # start right as the gathered K/V lands. The second AllGather's
        # sends need kt_own dt=2,3, so they're emitted after qproj(0) has
        # flushed the K3 rope.
        qproj_head(0)
        rope_finish()          # K3
        kv_sends(1, sendB, 2)
        nc.gpsimd.collective_compute(
            "AllGather", mybir.AluOpType.bypass,
            ins=[sendB.ap()], outs=[fullB.ap()], replica_groups=RG,
        )
        kv_loads(1, fullB)
        for i in range(1, H):
            qproj_head(i)
            rope_finish()      # Q[i-1]
        rope_finish()          # Q15
        for h in range(H):
            attn_head(h)

        # ---------- output projection ----------
        # wo streams through dead projection-phase weight tiles: wv_sb
        # (nt=0), wk2 (nt=1), and the wq pool's 8 bufs (nt=2,3) — all idle
        # after Q proj. The loads go on gpsimd: its DMA queues (166-168)
        # are disjoint from the rope-swap/weight-load queues, so a gated or
        # queued wo descriptor can never block the attention pipeline (the
        # old SP+time-gate scheme stalled PE 12.5us mid-attention). gpsimd
        # reaches these descriptors at ~155us, right after the second
        # AllGather's K/V loads; transfers finish long before out-proj.
        wo_sp = [wqp.tile([128, 4 * 512], bf, tag="wq", name=f"wosp{i}") for i in range(8)]

        def wo_slice(nt, ct):
            if nt == 0:
                return wv_sb[:, ct * 512:(ct + 1) * 512]
            if nt == 1:
                t = wk2[ct // 8]
                return t[:, (ct % 8) * 512:((ct % 8) + 1) * 512]
            t = wo_sp[(nt - 2) * 4 + ct // 4]
            return t[:, (ct % 4) * 512:((ct % 4) + 1) * 512]

        for nt in range(4):
            for ct in range(KS):
                nc.gpsimd.dma_start(wo_slice(nt, ct), wo_d.ap()[nt][:, ct * 512:(ct + 1) * 512])
        # head 15's epilogue: av tail + den folds now; the den matmul and
        # normalize are emitted INSIDE the first out-proj group (which runs
        # on a pp_st tile, so the pp_av rotation stays clobber-safe), after
        # ct=12 — by ct=15 the normalized a_sb[15] is ready.
        while avpend:
            avpend.pop(0)()
        dpend[0][0]()
        for nt in range(4):
            for qt in range(4):
                if nt == 0 and qt == 0:
                    pst = pp_st.tile([128, 3 * SQ], f32, tag="st", name="ops00")
                else:
                    pst = pp_av.tile([128, 512], f32, tag="av", name=f"ops{nt}_{qt}")
                for ct in range(KS):
                    nc.tensor.matmul(
                        pst[:, 0:512],
                        a_sb[:, ct * SQ + qt * 128: ct * SQ + qt * 128 + 128],
                        wo_slice(nt, ct),
                        start=(ct == 0), stop=(ct == KS - 1),
                    )
                    if nt == 0 and qt == 0 and ct == 12:
                        dpend.pop(0)[1]()   # den(15) + normalize
                ot = outp.tile([128, 512], bf, tag="ot")
                if nt == 3 and qt == 3:
                    # split the last tile so the final store overlaps the copy
                    for c0 in (0, 256):
                        nc.scalar.activation(ot[:, c0:c0 + 256], pst[:, c0:c0 + 256], AF.Copy)
                        nc.sync.dma_start(
                            out_d.ap()[qt * 128:(qt + 1) * 128, nt * 512 + c0:nt * 512 + c0 + 256],
                            ot[:, c0:c0 + 256],
                        )
                else:
                    nc.scalar.activation(ot[:], pst[:, 0:512], AF.Copy)
                    nc.sync.dma_start(out_d.ap()[qt * 128:(qt + 1) * 128, nt * 512:(nt + 1) * 512], ot[:])

    nc.compile()
    return nc


def get_nc():
    if "nc" not in _CACHE:
        _CACHE["nc"] = _build_nc()
    return _CACHE["nc"]


def make_in_maps(x, wq, bq, wk, bk, wv, bv, wo):
    bf16 = ml_dtypes.bfloat16
    perm = np.concatenate([np.arange(0, HD, 2), np.arange(1, HD, 2)])
    qcols = np.concatenate([h * HD + perm for h in range(H)])
    kcols = np.concatenate([h * HD + perm for h in range(KVH)])
    wq_p = wq[:, qcols]
    bq_p = np.ascontiguousarray(bq[qcols].reshape(H, HD).T).astype(np.float32)
    wk_p = wk[:, kcols]
    bk_p = np.ascontiguousarray(bk[kcols].reshape(KVH, HD).T).astype(np.float32)
    # pretile so every DMA is contiguous: wq [ht][p][ks][c], wk/wv [p][ks][c],
    # wo [nt][p][ct][c]
    wq_t = np.ascontiguousarray(
        wq_p.reshape(KS, 128, H, 128).transpose(2, 1, 0, 3).reshape(H, 128, KS * 128)
    ).astype(bf16)
    wk_t = np.ascontiguousarray(
        wk_p.reshape(KS, 128, DKV).transpose(1, 0, 2).reshape(128, KS * DKV)
    ).astype(bf16)
    wv_t = np.ascontiguousarray(
        wv.reshape(KS, 128, DKV).transpose(1, 0, 2).reshape(128, KS * DKV)
    ).astype(bf16)
    wo_t = np.ascontiguousarray(
        wo.reshape(KS, 128, 4, 512).transpose(2, 1, 0, 3).reshape(4, 128, KS * 512)
    ).astype(bf16)
    bv_rep = np.tile(bv.astype(np.float32), (128, 1))
    theta = (10000.0 ** (-np.arange(64, dtype=np.float64) / 64.0))
    ang = np.outer(np.arange(S, dtype=np.float64), theta)  # [S, 64]
    c = np.cos(ang).T.astype(np.float32)  # [64, S]
    s = np.sin(ang).T.astype(np.float32)
    cosT = np.concatenate([c, c], axis=0)      # [128, S]
    sinT = np.concatenate([-s, s], axis=0)     # [128, S]
    ones = np.ones((128, 128), dtype=bf16)
    # psw = perm.T @ qf must be the half-swap: psw[i] = qf[(i+64)%128]
    perm = np.roll(np.eye(128, dtype=np.float32), 64, axis=0).astype(bf16)

    in_maps = []
    for b in range(B):
        for g in range(G):
            sl = slice(g * SQ, (g + 1) * SQ)
            xt_c = np.ascontiguousarray(
                x[b, sl, :].T.reshape(KS, 128, SQ).transpose(1, 0, 2).reshape(128, KS * SQ)
            ).astype(bf16)
            in_maps.append({
                "xt": xt_c,
                "wq": wq_t, "wk": wk_t, "wv": wv_t, "wo": wo_t,
                "bq": bq_p, "bk": bk_p, "bv": bv_rep,
                "cosq": np.ascontiguousarray(cosT[:, sl]).astype(bf16),
                "sinq": np.ascontiguousarray(sinT[:, sl]).astype(bf16),
                "ones": ones,
                "perm": perm,
            })
    return in_maps


def assemble(results):
    out = np.empty((B, S, D), np.float32)
    for b in range(B):
        for g in range(G):
            out[b, g * SQ:(g + 1) * SQ, :] = results[b * G + g]["out"]
    return out


def kernel(x, wq, bq, wk, bk, wv, bv, wo):
    from concourse.bass_utils import run_bass_kernel_spmd

    x, wq, bq, wk, bk, wv, bv, wo = (
        np.asarray(t, dtype=np.float32) for t in (x, wq, bq, wk, bk, wv, bv, wo)
    )
    nc = get_nc()
    in_maps = make_in_maps(x, wq, bq, wk, bk, wv, bv, wo)
    # run twice and return the second result: the first execution after a
    # NEFF load has occasionally produced stale collective output.
    run_bass_kernel_spmd(nc, in_maps, core_ids=list(range(NCORES)))
    res = run_bass_kernel_spmd(nc, in_maps, core_ids=list(range(NCORES)))
    return assemble(res.results)



# revision 50
# speedup vs baseline: 1.0309x; 1.0166x over previous
"""Multi-head GQA attention (B=2, S=2048, D=2048, H=16, KVH=4) on 8 TRN2
NeuronCores.

Sharding: core i = (b, g) with b = i // 4 (batch), g = i % 4 (sequence
chunk of 512 queries). Each core computes Q for its 512 queries over all
16 heads, K/V for its own 512 sequence positions, AllGathers K/V within
its 4-core batch group, then runs full attention + output projection for
its query chunk. Host concatenates the 8 [512, 2048] chunks.

Layout strategy (no on-chip transposes):
 - host passes x transposed per chunk (xT [D, 512]) so projections
   computed as w.T @ xT yield QT/KT with head-dim on partitions —
   exactly the operand layout attention needs.
 - wq/wk columns permuted per head (even dims first, odd second) so RoPE
   halves are contiguous partition ranges [0:64)/[64:128). Scores are
   permutation-invariant since q and k are permuted identically.
 - scores computed transposed (ST[k, q] = KT.T @ QT) in k-tile GROUPS
   of (3,3,3,3,3,1) into [128,1536] 3-bank PSUM tiles, exp'd by ONE
   ScalarE activation per group straight out of PSUM (scale=1/sqrt(HD)
   folded in, no max-subtraction: scores are O(10) so f32 exp is
   safe). Wider groups amortize the 352-cycle ACT instruction overhead
   so the exp chain (~8.6us/head) sits just under the PE's ~8.8us.
 - softmax denominator: DVE accumulates the exp'd group tiles (bf16 2x
   mode), then ONE all-ones-stationary matmul per head replicates the
   denominator across partitions; normalization is reciprocal +
   elementwise multiply, no broadcast. The whole den/normalize chain
   of head h is deferred into head h+1's early groups so the PE never
   waits on the last exp + folds.
 - AV matmuls lag the exp pipeline by two groups; the last two AV
   groups of head h are emitted after head h+1's first score group.
 - RoPE's rotate-half swap is ONE permutation matmul per projection
   (psw = perm.T @ qf), emitted one head late so it never waits on the
   producing IDENTITY. (The old SBUF->SBUF swap DMAs shared hardware
   DMA queues with bulk weight loads and crawled whenever an AllGather
   transfer was in flight, stalling PE ~17us through a cross-engine
   WAR chain.)
 - PSUM (8 banks): st groups 2x3 banks; everything [128,512]-sized
   (projection accumulators as views of st tiles; av/psw/den/out-proj
   on the other pool's 2 banks).
 - weights are host-pretiled so every DMA is a contiguous block. The
   initial 6.4MB (wk/xt/wv + consts) is ONE SP descriptor stream in
   exact consumption order (finer chunks up front); wo streams through
   the dead wk/wv/wq tiles on gpsimd's queues (disjoint from the
   weight-load queues, so it can never block the attention pipeline).
 - startup: 5 dummy matmuls on a memset tile open the HAM activity
   window during the initial DMA wait (PE runs warm from ~11us instead
   of ~24us) and a dummy Exp preloads the ACT table set.
 - gathered-K/V loads on gpsimd (fire the moment the collective
   completes). The two AllGathers serialize on gpsimd; all 16 Q
   projections run before attention to cover that latency. Output is
   stored bf16 (the f32->bf16 round adds ~0.2% rms error; gate is 2%).
 - steady-state caveat: the board GPIO power throttle caps the PE at
   13/16 duty (~1.95GHz) from ~40us in, on all 8 cores. The kernel is
   ~92% PE-issue-bound at that clock; run-to-run variance is +-5us.
"""

import numpy as np
import ml_dtypes

B, S, D = 2, 2048, 2048
H, KVH = 16, 4
HD = D // H            # 128
R = H // KVH           # 4 (GQA repeat)
NCORES = 8
G = 4                  # cores per batch group = seq chunks
SQ = S // G            # 512 queries/keys per core chunk
DKV = KVH * HD         # 512
KS = D // 128          # 16 contraction slices
NKT = S // 128         # 16 key tiles
SCALE = 1.0 / float(np.sqrt(HD))

_CACHE = {}


def _build_nc():
    import concourse.tile as tile
    from concourse import bacc, mybir
    from contextlib import ExitStack

    f32 = mybir.dt.float32
    bf = mybir.dt.bfloat16
    AF = mybir.ActivationFunctionType

    nc = bacc.Bacc("TRN2", target_bir_lowering=False, debug=False, num_devices=NCORES)

    xt_d = nc.dram_tensor("xt", [128, KS * SQ], bf, kind="ExternalInput")
    wq_d = nc.dram_tensor("wq", [H, 128, KS * 128], bf, kind="ExternalInput")
    wk_d = nc.dram_tensor("wk", [128, KS * DKV], bf, kind="ExternalInput")
    wv_d = nc.dram_tensor("wv", [128, KS * DKV], bf, kind="ExternalInput")
    wo_d = nc.dram_tensor("wo", [4, 128, KS * 512], bf, kind="ExternalInput")
    bq_d = nc.dram_tensor("bq", [128, H], f32, kind="ExternalInput")
    bk_d = nc.dram_tensor("bk", [128, KVH], f32, kind="ExternalInput")
    bv_d = nc.dram_tensor("bv", [128, DKV], f32, kind="ExternalInput")
    cos_d = nc.dram_tensor("cosq", [128, SQ], bf, kind="ExternalInput")
    sin_d = nc.dram_tensor("sinq", [128, SQ], bf, kind="ExternalInput")
    ones_d = nc.dram_tensor("ones", [128, 128], bf, kind="ExternalInput")
    perm_d = nc.dram_tensor("perm", [128, 128], bf, kind="ExternalInput")
    out_d = nc.dram_tensor("out", [SQ, D], bf, kind="ExternalOutput")

    # two half-AllGathers, each carrying 2 kv heads' K and V (0.5MB/rank):
    # rows [0:256] = KT of the 2 heads, rows [256:512] = their V halves.
    sendA = nc.dram_tensor("sendA", [8, 128, 256], bf)
    sendB = nc.dram_tensor("sendB", [8, 128, 256], bf)
    fullA = nc.dram_tensor("fullA", [G * 8, 128, 256], bf)
    fullB = nc.dram_tensor("fullB", [G * 8, 128, 256], bf)
    RG = [[0, 1, 2, 3], [4, 5, 6, 7]]

    with tile.TileContext(nc) as tc, ExitStack() as ctx:
        const = ctx.enter_context(tc.tile_pool(name="const", bufs=1))
        big = ctx.enter_context(tc.tile_pool(name="big", bufs=1))
        wqp = ctx.enter_context(tc.tile_pool(name="wqp", bufs=8))
        fp = ctx.enter_context(tc.tile_pool(name="fp", bufs=4))
        rp = ctx.enter_context(tc.tile_pool(name="rp", bufs=4))
        ptp = ctx.enter_context(tc.tile_pool(name="ptp", bufs=6))
        sump = ctx.enter_context(tc.tile_pool(name="sump", bufs=3))
        outp = ctx.enter_context(tc.tile_pool(name="outp", bufs=2))
        recs = ctx.enter_context(tc.tile_pool(name="recs", bufs=2))
        # PSUM budget (8 banks x 2KB): st groups 2x3 banks + av/psw/den/out 2.
        # Projections allocate their [128,512] accumulators as views of the
        # 3-bank st tiles (the pools are static, so proj gets no pool of
        # its own); everything else [128,512]-sized shares pp_av.
        pp_st = ctx.enter_context(tc.tile_pool(name="pp_st", bufs=2, space="PSUM"))
        pp_av = ctx.enter_context(tc.tile_pool(name="pp_av", bufs=2, space="PSUM"))



        # ---------- warmup: HAM busy-window + ACT table preload ----------
        # the PE clock gate defaults to half rate until ~3.4us of sustained
        # activity; dummy matmuls on a memset tile start the busy window at
        # ~7us (during the initial DMA wait) so the real projections run
        # warm almost immediately. The dummy Exp loads the activation table
        # set (~1.3us) off the rope critical path.
        warm = const.tile([128, 512], bf)
        nc.vector.memset(warm[:], 0)
        wps = pp_av.tile([128, 512], f32, tag="av", name="warmps")
        for _ in range(5):
            nc.tensor.matmul(wps[:], warm[:, 0:128], warm[:], start=True, stop=True)
        dume = const.tile([128, 64], bf)
        nc.scalar.activation(dume[:], warm[:, 0:64], AF.Exp)

        # ---------- loads needed by the K/V path, first ----------
        # ONE SP stream in exact consumption order. Transfers drain the
        # shared DMA queues roughly in issue order, so anything issued
        # early that isn't needed early (v2 tried wv/xt tails on other
        # sequencers) steals bandwidth from the K-path slices and stalls
        # the first projections. Total here is ~6MB (~17us of HBM); the
        # K(0,1) sweep consumes its 4MB at about the rate it lands.
        def chunked_load(dst, src_ap, width, n=4, eng=None):
            step = width // n
            for j in range(n):
                e = eng if eng is not None else nc.sync
                e.dma_start(dst[:, j * step:(j + 1) * step], src_ap[:, j * step:(j + 1) * step])

        # xt/wk split into sub-tiles so the first projection chain starts
        # as soon as the first slices land (deps are view-overlap based).
        xt4 = [big.tile([128, 4 * SQ], bf, name=f"xt4_{i}") for i in range(4)]
        wk2 = [big.tile([128, 8 * DKV], bf, name=f"wk2_{i}") for i in range(2)]
        wv_sb = big.tile([128, KS * DKV], bf)
        cos_sb = const.tile([128, SQ], bf)
        sin_sb = const.tile([128, SQ], bf)
        bk_sb = const.tile([128, KVH], f32)
        bv_sb = const.tile([128, DKV], f32)
        perm_sb = const.tile([128, 128], bf)
        # one SP stream in global need order. (Tried alternating SP/ACT to
        # double descriptor issue rate: the ACT-issued descriptors' queue
        # waits occupy the ScalarE FIFO and starve the rope IDENTITYs —
        # +28us. Do NOT put bulk loads on ACT.)
        loads = [
            (wk2[0], (0, 512), wk_d), (xt4[0], (0, 512), xt_d),
            (xt4[0], (512, 1024), xt_d), (wk2[0], (512, 1280), wk_d),
            (xt4[0], (1024, 2048), xt_d), (wk2[0], (1280, 2304), wk_d),
            (wk2[0], (2304, 4096), wk_d),
            (xt4[1], (2048, 3072), xt_d), (xt4[1], (3072, 4096), xt_d),
            # V first halves next: the PE runs v_half(0) on these while
            # K's ks>=8 data (wk1/xt2/xt3) is still in flight. From here
            # the interleaved halves tolerate coarse waits, so use BIG
            # chunks: fewer descriptors = fewer per-queue issue bubbles =
            # higher aggregate DMA throughput.
            (wv_sb, (0, 2048), wv_d), (wv_sb, (2048, 4096), wv_d),
            (wk2[1], (4096, 5120), wk_d), (xt4[2], (4096, 5120), xt_d),
            (wk2[1], (5120, 6144), wk_d), (xt4[2], (5120, 6144), xt_d),
            (wk2[1], (6144, 7168), wk_d), (xt4[3], (6144, 7168), xt_d),
            (wk2[1], (7168, 8192), wk_d), (xt4[3], (7168, 8192), xt_d),
            # small consts: rope (cos/sin/perm) + biases feed the K->send
            # chain at ~30us; behind the whole 6MB they'd arrive too late.
            (cos_sb, None, cos_d), (sin_sb, None, sin_d),
            (bk_sb, None, bk_d), (bv_sb, None, bv_d), (perm_sb, None, perm_d),
            (wv_sb, (4096, 6144), wv_d), (wv_sb, (6144, 8192), wv_d),
        ]
        # (Tried offloading the xt tail + wv onto gpsimd's three idle DMA
        # queues for extra issue rate: consistently ~3us slower. Keep ONE
        # need-ordered SP stream.)
        base = {id(wk2[1]): 4096, id(xt4[1]): 2048, id(xt4[2]): 4096, id(xt4[3]): 6144}
        for dst, rng, src in loads:
            if rng is None:
                nc.sync.dma_start(dst[:], src.ap())
            else:
                off = base.get(id(dst), 0)
                nc.sync.dma_start(dst[:, rng[0] - off:rng[1] - off], src.ap()[:, rng[0]:rng[1]])

        def xts(ks):
            return xt4[ks // 4][:, (ks % 4) * SQ:(ks % 4 + 1) * SQ]

        # rope, rotate-half form: out = q*[cos;cos] + swap(q)*[-sin;sin].
        # The half-swap is ONE permutation matmul (perm_sb) instead of the
        # old pair of SBUF->SBUF DMAs: those descriptors shared hardware
        # DMA queues with the bulk weight loads, and whenever an AllGather
        # transfer was in flight they crawled, stalling the whole
        # scalar-FIFO -> PSUM-WAR chain behind them (2 stalls, ~17us).
        # rope_start runs on ScalarE right after the projection's last
        # matmul; rope_finish is emitted one head LATER so its P-matmul
        # never waits on the IDENTITY.
        pend = []

        def rope_start(ps_ap, bias_col, dst):
            qf = fp.tile([128, SQ], bf, tag="f")
            nc.scalar.activation(qf[:], ps_ap, AF.Identity, bias=bias_col)
            pend.append((qf, dst))

        def rope_finish():
            if not pend:
                return
            qf, dst = pend.pop(0)
            psw = pp_av.tile([128, SQ], f32, tag="av", name="psw")
            nc.tensor.matmul(psw[:], perm_sb[:], qf[:], start=True, stop=True)
            ta = rp.tile([128, SQ], bf, tag="rt")
            nc.vector.tensor_mul(ta[:], qf[:], cos_sb[:])
            tb = rp.tile([128, SQ], bf, tag="rt")
            nc.vector.tensor_mul(tb[:], psw[:], sin_sb[:])
            nc.vector.tensor_add(dst, ta[:], tb[:])

        # ---------- K/V projection for own chunk, RoPE(K), send ----------
        # order: K heads 0-1 -> V (all) -> AG1 fires early -> K heads 2-3 -> AG2
        kt_own = big.tile([128, KVH * SQ], bf)   # [p=hd, kv*SQ + s]
        v_own = big.tile([128, G * DKV], bf)     # [p=s%128, st*DKV + d]

        def kproj(dt):
            ps = pp_st.tile([128, 3 * SQ], f32, tag="st", name=f"kps{dt}")[:, 0:SQ]
            for ks in range(KS):
                nc.tensor.matmul(
                    ps,
                    wk2[ks // 8][:, (ks % 8) * DKV + dt * 128:(ks % 8) * DKV + (dt + 1) * 128],
                    xts(ks),
                    start=(ks == 0), stop=(ks == KS - 1),
                )
            rope_start(ps, bk_sb[:, dt:dt + 1], kt_own[:, dt * SQ:(dt + 1) * SQ])

        # separate gathered-KV tiles per AG pair (deps are tile-granular);
        # each pair's loads are emitted right after its AG trigger because the
        # collective instruction blocks the gpsimd engine until completion.
        ktfp = [big.tile([128, 2 * S], bf, name=f"ktf{p}") for p in range(2)]
        vfp = [big.tile([128, (G * G) * 256], bf, name=f"vf{p}") for p in range(2)]

        def kv_loads(pair, full_d):
            # on gpsimd: it has a dedicated DMA path and nothing else to do;
            # queued behind the collective they fire the moment it completes.
            # (On SP they'd sit behind the whole Q-proj descriptor chain.)
            ktf_t, vf_t = ktfp[pair], vfp[pair]
            for g in range(G):
                for hh in range(2):
                    for blk in range(2):
                        dst = ktf_t[:, hh * S + g * SQ + blk * 256: hh * S + g * SQ + (blk + 1) * 256]
                        nc.gpsimd.dma_start(dst, full_d.ap()[g * 8 + 2 * hh + blk])
                for st in range(G):
                    dst = vf_t[:, (g * G + st) * 256:(g * G + st) * 256 + 256]
                    nc.gpsimd.dma_start(dst, full_d.ap()[g * 8 + 4 + st])

        def kv_sends(pair, send_d, h0):
            # V halves packed as [128,256] blocks; layout is just bytes,
            # unpacked with matching APs on the receive side.
            for hh in range(2):
                for blk in range(2):
                    src = kt_own[:, (h0 + hh) * SQ + blk * 256:(h0 + hh) * SQ + (blk + 1) * 256]
                    nc.gpsimd.dma_start(send_d.ap()[2 * hh + blk], src)
            for st in range(G):
                src = v_own[:, st * DKV + pair * 256: st * DKV + pair * 256 + 256]
                nc.gpsimd.dma_start(send_d.ap()[4 + st], src)

        # K(0,1) and V interleaved at ks-HALF granularity, sharing the two
        # 3-bank st tiles (K heads in bank 0, two V st-chunks in banks 1-2
        # of each). The startup is DMA-paced; alternating K and V halves
        # in stream order gives the PE work during each data wait instead
        # of stalling at the wk1/xt2 (K's ks>=8) wall with V's data (wv
        # first half) already on chip.
        tA = pp_st.tile([128, 3 * SQ], f32, tag="st", name="kvA")
        tB = pp_st.tile([128, 3 * SQ], f32, tag="st", name="kvB")
        KT2 = (tA, tB)

        def k01_half(half):
            for ks in range(half * 8, half * 8 + 8):
                for dt in (0, 1):
                    nc.tensor.matmul(
                        KT2[dt][:, 0:SQ],
                        wk2[ks // 8][:, (ks % 8) * DKV + dt * 128:(ks % 8) * DKV + (dt + 1) * 128],
                        xts(ks),
                        start=(ks == 0), stop=(ks == KS - 1),
                    )

        def v_half(half):
            for ks in range(half * 8, half * 8 + 8):
                for st in range(G):
                    t = KT2[st // 2]
                    nc.tensor.matmul(
                        t[:, (1 + st % 2) * SQ:(2 + st % 2) * SQ],
                        xts(ks)[:, st * 128: st * 128 + 128],
                        wv_sb[:, ks * DKV:(ks + 1) * DKV],
                        start=(ks == 0), stop=(ks == KS - 1),
                    )

        k01_half(0)
        v_half(0)
        k01_half(1)
        rope_start(tA[:, 0:SQ], bk_sb[:, 0:1], kt_own[:, 0:SQ])
        rope_start(tB[:, 0:SQ], bk_sb[:, 1:2], kt_own[:, SQ:2 * SQ])
        v_half(1)
        rope_finish()          # K0 (IDENTITY ran during v_half(1))
        rope_finish()          # K1
        for st in range(G):
            t = KT2[st // 2]
            nc.vector.tensor_add(
                v_own[:, st * DKV:(st + 1) * DKV],
                t[:, (1 + st % 2) * SQ:(2 + st % 2) * SQ],
                bv_sb[:],
            )
        kv_sends(0, sendA, 0)
        nc.gpsimd.collective_compute(
            "AllGather", mybir.AluOpType.bypass,
            ins=[sendA.ap()], outs=[fullA.ap()], replica_groups=RG,
        )
        kv_loads(0, fullA)
        kproj(2)
        kproj(3)
        rope_finish()          # K2

        # ---------- remaining consts ----------
        bq_sb = const.tile([128, H], f32)
        ones_sb = const.tile([128, 128], bf)
        nc.sync.dma_start(bq_sb[:], bq_d.ap())
        nc.sync.dma_start(ones_sb[:], ones_d.ap())

        qt4 = [big.tile([128, 4 * SQ], bf, name=f"qt4_{i}") for i in range(4)]  # [p=hd, (h%4)*SQ + q]
        a_sb = big.tile([128, H * SQ], bf)       # [p=hd, h*SQ + q]  (AV^T, normalized)
        # k-tiles per exp group: 3-bank groups amortize the ScalarE ACTIVATE
        # overhead (352 cycles/instr); with pairs the exp chain (8x1147ns)
        # was the attention-phase critical path, above PE's 8.8us/head.
        GROUPS = (3, 3, 3, 3, 3, 1)
        GBASE = (0, 3, 6, 9, 12, 15)
        # deferred tails of the previous head, drained inside the next
        # head's early groups so the PE's score stream (which feeds the
        # exp pipeline) is never delayed by the previous head's epilogue:
        # avpend = its last two AV groups, dpend = (folds, den+normalize).
        avpend = []
        dpend = []

        def qproj_head(ht):
            wq_t = wqp.tile([128, KS * 128], bf, tag="wq")
            for j in range(4):
                nc.sync.dma_start(wq_t[:, j * 512:(j + 1) * 512], wq_d.ap()[ht][:, j * 512:(j + 1) * 512])
            ps = pp_st.tile([128, 3 * SQ], f32, tag="st", name=f"qps{ht}")[:, 0:SQ]
            for ks in range(KS):
                nc.tensor.matmul(
                    ps,
                    wq_t[:, ks * 128:(ks + 1) * 128],
                    xts(ks),
                    start=(ks == 0), stop=(ks == KS - 1),
                )
            rope_start(ps, bq_sb[:, ht:ht + 1], qt4[ht // 4][:, (ht % 4) * SQ:(ht % 4 + 1) * SQ])

        def attn_head(h):
            # scores + exp in k-tile GROUPS (3,3,3,3,2,2 over 3-bank PSUM
            # tiles): one ACTIVATE per group; softmax denominator via DVE
            # group-sums + ONE matmul per head, deferred into the NEXT
            # head's pipeline (emitted after its group-0 scores) so the PE
            # never waits on the last exp + fold chain. The den result goes
            # into the spare third bank of the head's final 2-wide group.
            kv = h // R
            ktf_t, vf_t = ktfp[kv // 2], vfp[kv // 2]
            kvh = kv % 2
            av = pp_av.tile([128, SQ], f32, tag="av", name=f"av{h}")
            ptsum = sump.tile([128, 3 * SQ], bf, tag="ptsum")
            pts = [None] * 6
            qsl = qt4[h // 4][:, (h % 4) * SQ:(h % 4 + 1) * SQ]

            def av_group(g):
                for j in range(GROUPS[g]):
                    kt = GBASE[g] + j
                    nc.tensor.matmul(
                        av[:],
                        vf_t[:, kt * 256 + kvh * 128: kt * 256 + (kvh + 1) * 128],
                        pts[g][:, j * SQ:(j + 1) * SQ],
                        start=(kt == 0), stop=(kt == NKT - 1),
                    )

            # AV runs two groups behind scores/exp so the PE never waits on
            # the exp of the group it just produced.
            for g in range(6):
                gsz = GROUPS[g]
                st_ps = pp_st.tile([128, 3 * SQ], f32, tag="st")
                for j in range(gsz):
                    kt = GBASE[g] + j
                    nc.tensor.matmul(
                        st_ps[:, j * SQ:(j + 1) * SQ],
                        ktf_t[:, kvh * S + kt * 128: kvh * S + (kt + 1) * 128],
                        qsl,
                        start=True, stop=True,
                    )
                if g == 0:
                    while avpend:
                        avpend.pop(0)()          # head h-1's av4/av5
                    if dpend:
                        dpend[0][0]()            # head h-1's den folds (DVE)
                elif g == 1 and dpend:
                    dpend.pop(0)[1]()            # head h-1's den/normalize
                w = gsz * SQ
                pt = ptp.tile([128, 3 * SQ], bf, tag="pt")
                nc.scalar.activation(pt[:, 0:w], st_ps[:, 0:w], AF.Exp, scale=SCALE)
                pts[g] = pt
                if g == 1:
                    nc.vector.tensor_add(ptsum[:], pts[0][:], pts[1][:])
                elif g > 1:
                    nc.vector.tensor_add(ptsum[:, 0:w], ptsum[:, 0:w], pt[:, 0:w])
                if g >= 2:
                    av_group(g - 2)
            avpend.append(lambda: av_group(4))
            avpend.append(lambda: av_group(5))

            # den tile allocated EAGERLY (keeps the av/den alternation on
            # pp_av's two bufs intact) but its instructions are deferred.
            den_t = pp_av.tile([128, SQ], f32, tag="av", name=f"den{h}")
            box = {}

            def den_folds(ptsum=ptsum, box=box):
                ps512 = sump.tile([128, SQ], bf, tag="ps512")
                nc.vector.tensor_add(ps512[:], ptsum[:, 0:SQ], ptsum[:, SQ:2 * SQ])
                nc.vector.tensor_add(ps512[:], ps512[:], ptsum[:, 2 * SQ:3 * SQ])
                box["ps512"] = ps512

            def den_norm(h=h, av=av, den_t=den_t, box=box):
                nc.tensor.matmul(den_t[:], ones_sb[:], box["ps512"][:], start=True, stop=True)
                recb = recs.tile([128, SQ], f32, tag="recb")
                nc.vector.reciprocal_approx_fast(recb[:], den_t[:])
                nc.vector.tensor_mul(a_sb[:, h * SQ:(h + 1) * SQ], av[:], recb[:])
            dpend.append((den_folds, den_norm))

        # ---------- schedule ----------
        # Q projections before attention: the PE work covers the first
        # AllGather's fire-to-complete latency, so attention heads 0-7
        # start right as the gathered K/V lands. The second AllGather's
        # sends need kt_own dt=2,3, so they're emitted after qproj(0) has
        # flushed the K3 rope.
        qproj_head(0)
        rope_finish()          # K3
        kv_sends(1, sendB, 2)
        nc.gpsimd.collective_compute(
            "AllGather", mybir.AluOpType.bypass,
            ins=[sendB.ap()], outs=[fullB.ap()], replica_groups=RG,
        )
        kv_loads(1, fullB)
        for i in range(1, H):
            qproj_head(i)
            rope_finish()      # Q[i-1]
        rope_finish()          # Q15
        for h in range(H):
            attn_head(h)

        # ---------- output projection ----------
        # wo streams through dead projection-phase weight tiles: wv_sb
        # (nt=0), wk2 (nt=1), and the wq pool's 8 bufs (nt=2,3) — all idle
        # after Q proj. The loads go on gpsimd: its DMA queues (166-168)
        # are disjoint from the rope-swap/weight-load queues, so a gated or
        # queued wo descriptor can never block the attention pipeline (the
        # old SP+time-gate scheme stalled PE 12.5us mid-attention). gpsimd
        # reaches these descriptors at ~155us, right after the second
        # AllGather's K/V loads; transfers finish long before out-proj.
        wo_sp = [wqp.tile([128, 4 * 512], bf, tag="wq", name=f"wosp{i}") for i in range(8)]

        def wo_slice(nt, ct):
            if nt == 0:
                return wv_sb[:, ct * 512:(ct + 1) * 512]
            if nt == 1:
                t = wk2[ct // 8]
                return t[:, (ct % 8) * 512:((ct % 8) + 1) * 512]
            t = wo_sp[(nt - 2) * 4 + ct // 4]
            return t[:, (ct % 4) * 512:((ct % 4) + 1) * 512]

        for nt in range(4):
            for ct in range(KS):
                nc.gpsimd.dma_start(wo_slice(nt, ct), wo_d.ap()[nt][:, ct * 512:(ct + 1) * 512])
        # head 15's epilogue: av tail + den folds now; the den matmul and
        # normalize are emitted INSIDE the first out-proj group (which runs
        # on a pp_st tile, so the pp_av rotation stays clobber-safe), after
        # ct=12 — by ct=15 the normalized a_sb[15] is ready.
        while avpend:
            avpend.pop(0)()
        dpend[0][0]()
        for nt in range(4):
            for qt in range(4):
                if nt == 0 and qt == 0:
                    pst = pp_st.tile([128, 3 * SQ], f32, tag="st", name="ops00")
                else:
                    pst = pp_av.tile([128, 512], f32, tag="av", name=f"ops{nt}_{qt}")
                for ct in range(KS):
                    nc.tensor.matmul(
                        pst[:, 0:512],
                        a_sb[:, ct * SQ + qt * 128: ct * SQ + qt * 128 + 128],
                        wo_slice(nt, ct),
                        start=(ct == 0), stop=(ct == KS - 1),
                    )
                    if nt == 0 and qt == 0 and ct == 12:
                        dpend.pop(0)[1]()   # den(15) + normalize
                ot = outp.tile([128, 512], bf, tag="ot")
                if nt == 3 and qt == 3:
                    # split the last tile so the final store overlaps the copy
                    for c0 in (0, 256):
                        nc.scalar.activation(ot[:, c0:c0 + 256], pst[:, c0:c0 + 256], AF.Copy)
                        nc.sync.dma_start(
                            out_d.ap()[qt * 128:(qt + 1) * 128, nt * 512 + c0:nt * 512 + c0 + 256],
                            ot[:, c0:c0 + 256],
                        )
                else:
                    nc.scalar.activation(ot[:], pst[:, 0:512], AF.Copy)
                    nc.sync.dma_start(out_d.ap()[qt * 128:(qt + 1) * 128, nt * 512:(nt + 1) * 512], ot[:])

    nc.compile()
    return nc


def get_nc():
    if "nc" not in _CACHE:
        _CACHE["nc"] = _build_nc()
    return _CACHE["nc"]


def make_in_maps(x, wq, bq, wk, bk, wv, bv, wo):
    bf16 = ml_dtypes.bfloat16
    perm = np.concatenate([np.arange(0, HD, 2), np.arange(1, HD, 2)])
    qcols = np.concatenate([h * HD + perm for h in range(H)])
    kcols = np.concatenate([h * HD + perm for h in range(KVH)])
    wq_p = wq[:, qcols]
    bq_p = np.ascontiguousarray(bq[qcols].reshape(H, HD).T).astype(np.float32)
    wk_p = wk[:, kcols]
    bk_p = np.ascontiguousarray(bk[kcols].reshape(KVH, HD).T).astype(np.float32)
    # pretile so every DMA is contiguous: wq [ht][p][ks][c], wk/wv [p][ks][c],
    # wo [nt][p][ct][c]
    wq_t = np.ascontiguousarray(
        wq_p.reshape(KS, 128, H, 128).transpose(2, 1, 0, 3).reshape(H, 128, KS * 128)
    ).astype(bf16)
    wk_t = np.ascontiguousarray(
        wk_p.reshape(KS, 128, DKV).transpose(1, 0, 2).reshape(128, KS * DKV)
    ).astype(bf16)
    wv_t = np.ascontiguousarray(
        wv.reshape(KS, 128, DKV).transpose(1, 0, 2).reshape(128, KS * DKV)
    ).astype(bf16)
    wo_t = np.ascontiguousarray(
        wo.reshape(KS, 128, 4, 512).transpose(2, 1, 0, 3).reshape(4, 128, KS * 512)
    ).astype(bf16)
    bv_rep = np.tile(bv.astype(np.float32), (128, 1))
    theta = (10000.0 ** (-np.arange(64, dtype=np.float64) / 64.0))
    ang = np.outer(np.arange(S, dtype=np.float64), theta)  # [S, 64]
    c = np.cos(ang).T.astype(np.float32)  # [64, S]
    s = np.sin(ang).T.astype(np.float32)
    cosT = np.concatenate([c, c], axis=0)      # [128, S]
    sinT = np.concatenate([-s, s], axis=0)     # [128, S]
    ones = np.ones((128, 128), dtype=bf16)
    # psw = perm.T @ qf must be the half-swap: psw[i] = qf[(i+64)%128]
    perm = np.roll(np.eye(128, dtype=np.float32), 64, axis=0).astype(bf16)

    in_maps = []
    for b in range(B):
        for g in range(G):
            sl = slice(g * SQ, (g + 1) * SQ)
            xt_c = np.ascontiguousarray(
                x[b, sl, :].T.reshape(KS, 128, SQ).transpose(1, 0, 2).reshape(128, KS * SQ)
            ).astype(bf16)
            in_maps.append({
                "xt": xt_c,
                "wq": wq_t, "wk": wk_t, "wv": wv_t, "wo": wo_t,
                "bq": bq_p, "bk": bk_p, "bv": bv_rep,
                "cosq": np.ascontiguousarray(cosT[:, sl]).astype(bf16),
                "sinq": np.ascontiguousarray(sinT[:, sl]).astype(bf16),
                "ones": ones,
                "perm": perm,
            })
    return in_maps


def assemble(results):
    out = np.empty((B, S, D), np.float32)
    for b in range(B):
        for g in range(G):
            out[b, g * SQ:(g + 1) * SQ, :] = results[b * G + g]["out"]
    return out


def kernel(x, wq, bq, wk, bk, wv, bv, wo):
    from concourse.bass_utils import run_bass_kernel_spmd

    x, wq, bq, wk, bk, wv, bv, wo = (
        np.asarray(t, dtype=np.float32) for t in (x, wq, bq, wk, bk, wv, bv, wo)
    )
    nc = get_nc()
    in_maps = make_in_maps(x, wq, bq, wk, bk, wv, bv, wo)
    # run twice and return the second result: the first execution after a
    # NEFF load has occasionally produced stale collective output.
    run_bass_kernel_spmd(nc, in_maps, core_ids=list(range(NCORES)))
    res = run_bass_kernel_spmd(nc, in_maps, core_ids=list(range(NCORES)))
    return assemble(res.results)



# revision 54
# speedup vs baseline: 1.0376x; 1.0065x over previous
"""Multi-head GQA attention (B=2, S=2048, D=2048, H=16, KVH=4) on 8 TRN2
NeuronCores.

Sharding: core i = (b, g) with b = i // 4 (batch), g = i % 4 (sequence
chunk of 512 queries). Each core computes Q for its 512 queries over all
16 heads, K/V for its own 512 sequence positions, AllGathers K/V within
its 4-core batch group, then runs full attention + output projection for
its query chunk. Host concatenates the 8 [512, 2048] chunks.

Layout strategy (no on-chip transposes):
 - host passes x transposed per chunk (xT [D, 512]) so projections
   computed as w.T @ xT yield QT/KT with head-dim on partitions —
   exactly the operand layout attention needs.
 - wq/wk columns permuted per head (even dims first, odd second) so RoPE
   halves are contiguous partition ranges [0:64)/[64:128). Scores are
   permutation-invariant since q and k are permuted identically.
 - scores computed transposed (ST[k, q] = KT.T @ QT) in k-tile GROUPS
   of (3,3,3,3,3,1) into [128,1536] 3-bank PSUM tiles, exp'd by ONE
   ScalarE activation per group straight out of PSUM (scale=1/sqrt(HD)
   folded in, no max-subtraction: scores are O(10) so f32 exp is
   safe). Wider groups amortize the 352-cycle ACT instruction overhead
   so the exp chain (~8.6us/head) sits just under the PE's ~8.8us.
 - softmax denominator: DVE accumulates the exp'd group tiles (bf16 2x
   mode), then ONE all-ones-stationary matmul per head replicates the
   denominator across partitions; normalization is reciprocal +
   elementwise multiply, no broadcast. The whole den/normalize chain
   of head h is deferred into head h+1's early groups so the PE never
   waits on the last exp + folds.
 - AV matmuls lag the exp pipeline by two groups; the last two AV
   groups of head h are emitted after head h+1's first score group.
 - RoPE's rotate-half swap is ONE permutation matmul per projection
   (psw = perm.T @ qf), emitted one head late so it never waits on the
   producing IDENTITY. (The old SBUF->SBUF swap DMAs shared hardware
   DMA queues with bulk weight loads and crawled whenever an AllGather
   transfer was in flight, stalling PE ~17us through a cross-engine
   WAR chain.)
 - PSUM (8 banks): st groups 2x3 banks; everything [128,512]-sized
   (projection accumulators as views of st tiles; av/psw/den/out-proj
   on the other pool's 2 banks).
 - weights are host-pretiled so every DMA is a contiguous block. The
   initial 6.4MB (wk/xt/wv + consts) is ONE SP descriptor stream in
   exact consumption order (finer chunks up front); wo streams through
   the dead wk/wv/wq tiles on gpsimd's queues (disjoint from the
   weight-load queues, so it can never block the attention pipeline).
 - startup: 5 dummy matmuls on a memset tile open the HAM activity
   window during the initial DMA wait (PE runs warm from ~11us instead
   of ~24us) and a dummy Exp preloads the ACT table set.
 - gathered-K/V loads on gpsimd (fire the moment the collective
   completes). The two AllGathers serialize on gpsimd; all 16 Q
   projections run before attention to cover that latency. Output is
   stored bf16 (the f32->bf16 round adds ~0.2% rms error; gate is 2%).
 - steady-state caveat: the board GPIO power throttle caps the PE at
   13/16 duty (~1.95GHz) from ~40us in, on all 8 cores. The kernel is
   ~92% PE-issue-bound at that clock; run-to-run variance is +-5us.
"""

import numpy as np
import ml_dtypes

B, S, D = 2, 2048, 2048
H, KVH = 16, 4
HD = D // H            # 128
R = H // KVH           # 4 (GQA repeat)
NCORES = 8
G = 4                  # cores per batch group = seq chunks
SQ = S // G            # 512 queries/keys per core chunk
DKV = KVH * HD         # 512
KS = D // 128          # 16 contraction slices
NKT = S // 128         # 16 key tiles
SCALE = 1.0 / float(np.sqrt(HD))

_CACHE = {}


def _build_nc():
    import concourse.tile as tile
    from concourse import bacc, mybir
    from contextlib import ExitStack

    f32 = mybir.dt.float32
    bf = mybir.dt.bfloat16
    AF = mybir.ActivationFunctionType

    nc = bacc.Bacc("TRN2", target_bir_lowering=False, debug=False, num_devices=NCORES)

    xt_d = nc.dram_tensor("xt", [128, KS * SQ], bf, kind="ExternalInput")
    wq_d = nc.dram_tensor("wq", [H, 128, KS * 128], bf, kind="ExternalInput")
    wk_d = nc.dram_tensor("wk", [128, KS * DKV], bf, kind="ExternalInput")
    wv_d = nc.dram_tensor("wv", [128, KS * DKV], bf, kind="ExternalInput")
    wo_d = nc.dram_tensor("wo", [4, 128, KS * 512], bf, kind="ExternalInput")
    bq_d = nc.dram_tensor("bq", [128, H], f32, kind="ExternalInput")
    bk_d = nc.dram_tensor("bk", [128, KVH], f32, kind="ExternalInput")
    bv_d = nc.dram_tensor("bv", [128, DKV], f32, kind="ExternalInput")
    cos_d = nc.dram_tensor("cosq", [128, SQ], bf, kind="ExternalInput")
    sin_d = nc.dram_tensor("sinq", [128, SQ], bf, kind="ExternalInput")
    ones_d = nc.dram_tensor("ones", [128, 128], bf, kind="ExternalInput")
    perm_d = nc.dram_tensor("perm", [128, 128], bf, kind="ExternalInput")
    out_d = nc.dram_tensor("out", [SQ, D], bf, kind="ExternalOutput")

    # two half-AllGathers, each carrying 2 kv heads' K and V (0.5MB/rank):
    # rows [0:256] = KT of the 2 heads, rows [256:512] = their V halves.
    sendA = nc.dram_tensor("sendA", [8, 128, 256], bf)
    sendB = nc.dram_tensor("sendB", [8, 128, 256], bf)
    fullA = nc.dram_tensor("fullA", [G * 8, 128, 256], bf)
    fullB = nc.dram_tensor("fullB", [G * 8, 128, 256], bf)
    RG = [[0, 1, 2, 3], [4, 5, 6, 7]]

    with tile.TileContext(nc) as tc, ExitStack() as ctx:
        const = ctx.enter_context(tc.tile_pool(name="const", bufs=1))
        big = ctx.enter_context(tc.tile_pool(name="big", bufs=1))
        wqp = ctx.enter_context(tc.tile_pool(name="wqp", bufs=8))
        fp = ctx.enter_context(tc.tile_pool(name="fp", bufs=4))
        rp = ctx.enter_context(tc.tile_pool(name="rp", bufs=4))
        ptp = ctx.enter_context(tc.tile_pool(name="ptp", bufs=6))
        sump = ctx.enter_context(tc.tile_pool(name="sump", bufs=3))
        outp = ctx.enter_context(tc.tile_pool(name="outp", bufs=2))
        recs = ctx.enter_context(tc.tile_pool(name="recs", bufs=2))
        # PSUM budget (8 banks x 2KB): st groups 2x3 banks + av/psw/den/out 2.
        # Projections allocate their [128,512] accumulators as views of the
        # 3-bank st tiles (the pools are static, so proj gets no pool of
        # its own); everything else [128,512]-sized shares pp_av.
        pp_st = ctx.enter_context(tc.tile_pool(name="pp_st", bufs=2, space="PSUM"))
        pp_av = ctx.enter_context(tc.tile_pool(name="pp_av", bufs=2, space="PSUM"))



        # ---------- warmup: HAM busy-window + ACT table preload ----------
        # the PE clock gate defaults to half rate until ~3.4us of sustained
        # activity; dummy matmuls on a memset tile start the busy window at
        # ~7us (during the initial DMA wait) so the real projections run
        # warm almost immediately. The dummy Exp loads the activation table
        # set (~1.3us) off the rope critical path.
        warm = const.tile([128, 512], bf)
        nc.vector.memset(warm[:], 0)
        wps = pp_av.tile([128, 512], f32, tag="av", name="warmps")
        for _ in range(5):
            nc.tensor.matmul(wps[:], warm[:, 0:128], warm[:], start=True, stop=True)
        dume = const.tile([128, 64], bf)
        nc.scalar.activation(dume[:], warm[:, 0:64], AF.Exp)

        # ---------- loads needed by the K/V path, first ----------
        # ONE SP stream in exact consumption order. Transfers drain the
        # shared DMA queues roughly in issue order, so anything issued
        # early that isn't needed early (v2 tried wv/xt tails on other
        # sequencers) steals bandwidth from the K-path slices and stalls
        # the first projections. Total here is ~6MB (~17us of HBM); the
        # K(0,1) sweep consumes its 4MB at about the rate it lands.
        def chunked_load(dst, src_ap, width, n=4, eng=None):
            step = width // n
            for j in range(n):
                e = eng if eng is not None else nc.sync
                e.dma_start(dst[:, j * step:(j + 1) * step], src_ap[:, j * step:(j + 1) * step])

        # xt/wk split into sub-tiles so the first projection chain starts
        # as soon as the first slices land (deps are view-overlap based).
        xt4 = [big.tile([128, 4 * SQ], bf, name=f"xt4_{i}") for i in range(4)]
        wk2 = [big.tile([128, 8 * DKV], bf, name=f"wk2_{i}") for i in range(2)]
        wv_sb = big.tile([128, KS * DKV], bf)
        cos_sb = const.tile([128, SQ], bf)
        sin_sb = const.tile([128, SQ], bf)
        bk_sb = const.tile([128, KVH], f32)
        bv_sb = const.tile([128, DKV], f32)
        perm_sb = const.tile([128, 128], bf)
        # one SP stream in global need order. (Tried alternating SP/ACT to
        # double descriptor issue rate: the ACT-issued descriptors' queue
        # waits occupy the ScalarE FIFO and starve the rope IDENTITYs —
        # +28us. Do NOT put bulk loads on ACT.)
        loads = [
            (wk2[0], (0, 512), wk_d), (xt4[0], (0, 512), xt_d),
            (xt4[0], (512, 1024), xt_d), (wk2[0], (512, 1280), wk_d),
            (xt4[0], (1024, 2048), xt_d), (wk2[0], (1280, 2304), wk_d),
            (wk2[0], (2304, 4096), wk_d),
            (xt4[1], (2048, 3072), xt_d), (xt4[1], (3072, 4096), xt_d),
            # V first halves next: the PE runs v_half(0) on these while
            # K's ks>=8 data (wk1/xt2/xt3) is still in flight.
            (wv_sb, (0, 2048), wv_d), (wv_sb, (2048, 4096), wv_d),
            (wk2[1], (4096, 5120), wk_d), (xt4[2], (4096, 5120), xt_d),
            (wk2[1], (5120, 6144), wk_d), (xt4[2], (5120, 6144), xt_d),
            (wk2[1], (6144, 7168), wk_d), (xt4[3], (6144, 7168), xt_d),
            (wk2[1], (7168, 8192), wk_d), (xt4[3], (7168, 8192), xt_d),
            # small consts: rope (cos/sin/perm) + biases feed the K->send
            # chain at ~30us. NOTE: tried moving these earlier in the SP
            # stream (+30us, queue-rotation wreck) and onto gpsimd's idle
            # queues (+4us median). This position measured best.
            (cos_sb, None, cos_d), (sin_sb, None, sin_d),
            (bk_sb, None, bk_d), (bv_sb, None, bv_d), (perm_sb, None, perm_d),
            (wv_sb, (4096, 6144), wv_d), (wv_sb, (6144, 8192), wv_d),
        ]
        # (Tried offloading the xt tail + wv onto gpsimd's three idle DMA
        # queues for extra issue rate: consistently ~3us slower. Keep ONE
        # need-ordered SP stream.)
        base = {id(wk2[1]): 4096, id(xt4[1]): 2048, id(xt4[2]): 4096, id(xt4[3]): 6144}
        for dst, rng, src in loads:
            if rng is None:
                nc.sync.dma_start(dst[:], src.ap())
            else:
                off = base.get(id(dst), 0)
                nc.sync.dma_start(dst[:, rng[0] - off:rng[1] - off], src.ap()[:, rng[0]:rng[1]])

        def xts(ks):
            return xt4[ks // 4][:, (ks % 4) * SQ:(ks % 4 + 1) * SQ]

        # rope, rotate-half form: out = q*[cos;cos] + swap(q)*[-sin;sin].
        # The half-swap is ONE permutation matmul (perm_sb) instead of the
        # old pair of SBUF->SBUF DMAs: those descriptors shared hardware
        # DMA queues with the bulk weight loads, and whenever an AllGather
        # transfer was in flight they crawled, stalling the whole
        # scalar-FIFO -> PSUM-WAR chain behind them (2 stalls, ~17us).
        # rope_start runs on ScalarE right after the projection's last
        # matmul; rope_finish is emitted one head LATER so its P-matmul
        # never waits on the IDENTITY.
        pend = []

        def rope_start(ps_ap, bias_col, dst):
            qf = fp.tile([128, SQ], bf, tag="f")
            nc.scalar.activation(qf[:], ps_ap, AF.Identity, bias=bias_col)
            pend.append((qf, dst))

        def rope_finish():
            if not pend:
                return
            qf, dst = pend.pop(0)
            psw = pp_av.tile([128, SQ], f32, tag="av", name="psw")
            nc.tensor.matmul(psw[:], perm_sb[:], qf[:], start=True, stop=True)
            ta = rp.tile([128, SQ], bf, tag="rt")
            nc.vector.tensor_mul(ta[:], qf[:], cos_sb[:])
            tb = rp.tile([128, SQ], bf, tag="rt")
            nc.vector.tensor_mul(tb[:], psw[:], sin_sb[:])
            nc.vector.tensor_add(dst, ta[:], tb[:])

        # ---------- K/V projection for own chunk, RoPE(K), send ----------
        # order: K heads 0-1 -> V (all) -> AG1 fires early -> K heads 2-3 -> AG2
        kt_own = big.tile([128, KVH * SQ], bf)   # [p=hd, kv*SQ + s]
        v_own = big.tile([128, G * DKV], bf)     # [p=s%128, st*DKV + d]

        def kproj(dt):
            ps = pp_st.tile([128, 3 * SQ], f32, tag="st", name=f"kps{dt}")[:, 0:SQ]
            for ks in range(KS):
                nc.tensor.matmul(
                    ps,
                    wk2[ks // 8][:, (ks % 8) * DKV + dt * 128:(ks % 8) * DKV + (dt + 1) * 128],
                    xts(ks),
                    start=(ks == 0), stop=(ks == KS - 1),
                )
            rope_start(ps, bk_sb[:, dt:dt + 1], kt_own[:, dt * SQ:(dt + 1) * SQ])

        # separate gathered-KV tiles per AG pair (deps are tile-granular);
        # each pair's loads are emitted right after its AG trigger because the
        # collective instruction blocks the gpsimd engine until completion.
        ktfp = [big.tile([128, 2 * S], bf, name=f"ktf{p}") for p in range(2)]
        vfp = [big.tile([128, (G * G) * 256], bf, name=f"vf{p}") for p in range(2)]

        def kv_loads(pair, full_d):
            # on gpsimd: it has a dedicated DMA path and nothing else to do;
            # queued behind the collective they fire the moment it completes.
            # (On SP they'd sit behind the whole Q-proj descriptor chain.)
            ktf_t, vf_t = ktfp[pair], vfp[pair]
            for g in range(G):
                for hh in range(2):
                    for blk in range(2):
                        dst = ktf_t[:, hh * S + g * SQ + blk * 256: hh * S + g * SQ + (blk + 1) * 256]
                        nc.gpsimd.dma_start(dst, full_d.ap()[g * 8 + 2 * hh + blk])
                for st in range(G):
                    dst = vf_t[:, (g * G + st) * 256:(g * G + st) * 256 + 256]
                    nc.gpsimd.dma_start(dst, full_d.ap()[g * 8 + 4 + st])

        def kv_sends(pair, send_d, h0):
            # V halves packed as [128,256] blocks; layout is just bytes,
            # unpacked with matching APs on the receive side.
            for hh in range(2):
                for blk in range(2):
                    src = kt_own[:, (h0 + hh) * SQ + blk * 256:(h0 + hh) * SQ + (blk + 1) * 256]
                    nc.gpsimd.dma_start(send_d.ap()[2 * hh + blk], src)
            for st in range(G):
                src = v_own[:, st * DKV + pair * 256: st * DKV + pair * 256 + 256]
                nc.gpsimd.dma_start(send_d.ap()[4 + st], src)

        # K(0,1) and V interleaved at ks-HALF granularity, sharing the two
        # 3-bank st tiles (K heads in bank 0, two V st-chunks in banks 1-2
        # of each). The startup is DMA-paced; alternating K and V halves
        # in stream order gives the PE work during each data wait instead
        # of stalling at the wk1/xt2 (K's ks>=8) wall with V's data (wv
        # first half) already on chip.
        tA = pp_st.tile([128, 3 * SQ], f32, tag="st", name="kvA")
        tB = pp_st.tile([128, 3 * SQ], f32, tag="st", name="kvB")
        KT2 = (tA, tB)

        def k01_half(half):
            for ks in range(half * 8, half * 8 + 8):
                for dt in (0, 1):
                    nc.tensor.matmul(
                        KT2[dt][:, 0:SQ],
                        wk2[ks // 8][:, (ks % 8) * DKV + dt * 128:(ks % 8) * DKV + (dt + 1) * 128],
                        xts(ks),
                        start=(ks == 0), stop=(ks == KS - 1),
                    )

        def v_half(half):
            for ks in range(half * 8, half * 8 + 8):
                for st in range(G):
                    t = KT2[st // 2]
                    nc.tensor.matmul(
                        t[:, (1 + st % 2) * SQ:(2 + st % 2) * SQ],
                        xts(ks)[:, st * 128: st * 128 + 128],
                        wv_sb[:, ks * DKV:(ks + 1) * DKV],
                        start=(ks == 0), stop=(ks == KS - 1),
                    )

        k01_half(0)
        v_half(0)
        k01_half(1)
        rope_start(tA[:, 0:SQ], bk_sb[:, 0:1], kt_own[:, 0:SQ])
        rope_start(tB[:, 0:SQ], bk_sb[:, 1:2], kt_own[:, SQ:2 * SQ])
        v_half(1)
        rope_finish()          # K0 (IDENTITY ran during v_half(1))
        rope_finish()          # K1
        for st in range(G):
            t = KT2[st // 2]
            nc.vector.tensor_add(
                v_own[:, st * DKV:(st + 1) * DKV],
                t[:, (1 + st % 2) * SQ:(2 + st % 2) * SQ],
                bv_sb[:],
            )
        kv_sends(0, sendA, 0)
        nc.gpsimd.collective_compute(
            "AllGather", mybir.AluOpType.bypass,
            ins=[sendA.ap()], outs=[fullA.ap()], replica_groups=RG,
        )
        kv_loads(0, fullA)
        kproj(2)
        kproj(3)
        rope_finish()          # K2

        # ---------- remaining consts ----------
        bq_sb = const.tile([128, H], f32)
        ones_sb = const.tile([128, 128], bf)
        nc.sync.dma_start(bq_sb[:], bq_d.ap())
        nc.sync.dma_start(ones_sb[:], ones_d.ap())

        qt4 = [big.tile([128, 4 * SQ], bf, name=f"qt4_{i}") for i in range(4)]  # [p=hd, (h%4)*SQ + q]
        a_sb = big.tile([128, H * SQ], bf)       # [p=hd, h*SQ + q]  (AV^T, normalized)
        # k-tiles per exp group: 3-bank groups amortize the ScalarE ACTIVATE
        # overhead (352 cycles/instr); with pairs the exp chain (8x1147ns)
        # was the attention-phase critical path, above PE's 8.8us/head.
        GROUPS = (3, 3, 3, 3, 3, 1)
        GBASE = (0, 3, 6, 9, 12, 15)
        # deferred tails of the previous head, drained inside the next
        # head's early groups so the PE's score stream (which feeds the
        # exp pipeline) is never delayed by the previous head's epilogue:
        # avpend = its last two AV groups, dpend = (folds, den+normalize).
        avpend = []
        dpend = []

        def qproj_head(ht):
            wq_t = wqp.tile([128, KS * 128], bf, tag="wq")
            for j in range(4):
                nc.sync.dma_start(wq_t[:, j * 512:(j + 1) * 512], wq_d.ap()[ht][:, j * 512:(j + 1) * 512])
            ps = pp_st.tile([128, 3 * SQ], f32, tag="st", name=f"qps{ht}")[:, 0:SQ]
            for ks in range(KS):
                nc.tensor.matmul(
                    ps,
                    wq_t[:, ks * 128:(ks + 1) * 128],
                    xts(ks),
                    start=(ks == 0), stop=(ks == KS - 1),
                )
            rope_start(ps, bq_sb[:, ht:ht + 1], qt4[ht // 4][:, (ht % 4) * SQ:(ht % 4 + 1) * SQ])

        def attn_head(h):
            # scores + exp in k-tile GROUPS (3,3,3,3,2,2 over 3-bank PSUM
            # tiles): one ACTIVATE per group; softmax denominator via DVE
            # group-sums + ONE matmul per head, deferred into the NEXT
            # head's pipeline (emitted after its group-0 scores) so the PE
            # never waits on the last exp + fold chain. The den result goes
            # into the spare third bank of the head's final 2-wide group.
            kv = h // R
            ktf_t, vf_t = ktfp[kv // 2], vfp[kv // 2]
            kvh = kv % 2
            av = pp_av.tile([128, SQ], f32, tag="av", name=f"av{h}")
            ptsum = sump.tile([128, 3 * SQ], bf, tag="ptsum")
            pts = [None] * 6
            qsl = qt4[h // 4][:, (h % 4) * SQ:(h % 4 + 1) * SQ]

            def av_group(g):
                for j in range(GROUPS[g]):
                    kt = GBASE[g] + j
                    nc.tensor.matmul(
                        av[:],
                        vf_t[:, kt * 256 + kvh * 128: kt * 256 + (kvh + 1) * 128],
                        pts[g][:, j * SQ:(j + 1) * SQ],
                        start=(kt == 0), stop=(kt == NKT - 1),
                    )

            # AV runs two groups behind scores/exp so the PE never waits on
            # the exp of the group it just produced.
            for g in range(6):
                gsz = GROUPS[g]
                st_ps = pp_st.tile([128, 3 * SQ], f32, tag="st")
                for j in range(gsz):
                    kt = GBASE[g] + j
                    nc.tensor.matmul(
                        st_ps[:, j * SQ:(j + 1) * SQ],
                        ktf_t[:, kvh * S + kt * 128: kvh * S + (kt + 1) * 128],
                        qsl,
                        start=True, stop=True,
                    )
                if g == 0:
                    while avpend:
                        avpend.pop(0)()          # head h-1's av4/av5
                    if dpend:
                        dpend[0][0]()            # head h-1's den folds (DVE)
                elif g == 1 and dpend:
                    dpend.pop(0)[1]()            # head h-1's den/normalize
                w = gsz * SQ
                pt = ptp.tile([128, 3 * SQ], bf, tag="pt")
                nc.scalar.activation(pt[:, 0:w], st_ps[:, 0:w], AF.Exp, scale=SCALE)
                pts[g] = pt
                if g == 1:
                    nc.vector.tensor_add(ptsum[:], pts[0][:], pts[1][:])
                elif g > 1:
                    nc.vector.tensor_add(ptsum[:, 0:w], ptsum[:, 0:w], pt[:, 0:w])
                if g >= 2:
                    av_group(g - 2)
            avpend.append(lambda: av_group(4))
            avpend.append(lambda: av_group(5))

            # den tile allocated EAGERLY (keeps the av/den alternation on
            # pp_av's two bufs intact) but its instructions are deferred.
            den_t = pp_av.tile([128, SQ], f32, tag="av", name=f"den{h}")
            box = {}

            def den_folds(ptsum=ptsum, box=box):
                ps512 = sump.tile([128, SQ], bf, tag="ps512")
                nc.vector.tensor_add(ps512[:], ptsum[:, 0:SQ], ptsum[:, SQ:2 * SQ])
                nc.vector.tensor_add(ps512[:], ps512[:], ptsum[:, 2 * SQ:3 * SQ])
                box["ps512"] = ps512

            def den_norm(h=h, av=av, den_t=den_t, box=box):
                nc.tensor.matmul(den_t[:], ones_sb[:], box["ps512"][:], start=True, stop=True)
                recb = recs.tile([128, SQ], f32, tag="recb")
                nc.vector.reciprocal_approx_fast(recb[:], den_t[:])
                nc.vector.tensor_mul(a_sb[:, h * SQ:(h + 1) * SQ], av[:], recb[:])
            dpend.append((den_folds, den_norm))

        # ---------- schedule ----------
        # Q projections before attention: the PE work covers the first
        # AllGather's fire-to-complete latency, so attention heads 0-7
        # start right as the gathered K/V lands. The second AllGather's
        # sends need kt_own dt=2,3, so they're emitted after qproj(0) has
        # flushed the K3 rope.
        qproj_head(0)
        rope_finish()          # K3
        kv_sends(1, sendB, 2)
        nc.gpsimd.collective_compute(
            "AllGather", mybir.AluOpType.bypass,
            ins=[sendB.ap()], outs=[fullB.ap()], replica_groups=RG,
        )
        kv_loads(1, fullB)
        for i in range(1, H):
            qproj_head(i)
            rope_finish()      # Q[i-1]
        rope_finish()          # Q15
        for h in range(H):
            attn_head(h)

        # ---------- output projection ----------
        # wo streams through dead projection-phase weight tiles: wv_sb
        # (nt=0), wk2 (nt=1), and the wq pool's 8 bufs (nt=2,3) — all idle
        # after Q proj. The loads go on gpsimd: its DMA queues (166-168)
        # are disjoint from the rope-swap/weight-load queues, so a gated or
        # queued wo descriptor can never block the attention pipeline (the
        # old SP+time-gate scheme stalled PE 12.5us mid-attention). gpsimd
        # reaches these descriptors at ~155us, right after the second
        # AllGather's K/V loads; transfers finish long before out-proj.
        wo_sp = [wqp.tile([128, 4 * 512], bf, tag="wq", name=f"wosp{i}") for i in range(8)]

        def wo_slice(nt, ct):
            if nt == 0:
                return wv_sb[:, ct * 512:(ct + 1) * 512]
            if nt == 1:
                t = wk2[ct // 8]
                return t[:, (ct % 8) * 512:((ct % 8) + 1) * 512]
            t = wo_sp[(nt - 2) * 4 + ct // 4]
            return t[:, (ct % 4) * 512:((ct % 4) + 1) * 512]

        for nt in range(4):
            for ct in range(KS):
                nc.gpsimd.dma_start(wo_slice(nt, ct), wo_d.ap()[nt][:, ct * 512:(ct + 1) * 512])
        # head 15's epilogue: av tail + den folds now; the den matmul and
        # normalize are emitted INSIDE the first out-proj group (which runs
        # on a pp_st tile, so the pp_av rotation stays clobber-safe), after
        # ct=12 — by ct=15 the normalized a_sb[15] is ready.
        while avpend:
            avpend.pop(0)()
        dpend[0][0]()
        for nt in range(4):
            for qt in range(4):
                if nt == 0 and qt == 0:
                    pst = pp_st.tile([128, 3 * SQ], f32, tag="st", name="ops00")
                else:
                    pst = pp_av.tile([128, 512], f32, tag="av", name=f"ops{nt}_{qt}")
                for ct in range(KS):
                    nc.tensor.matmul(
                        pst[:, 0:512],
                        a_sb[:, ct * SQ + qt * 128: ct * SQ + qt * 128 + 128],
                        wo_slice(nt, ct),
                        start=(ct == 0), stop=(ct == KS - 1),
                    )
                    if nt == 0 and qt == 0 and ct == 12:
                        dpend.pop(0)[1]()   # den(15) + normalize
                ot = outp.tile([128, 512], bf, tag="ot")
                if nt == 3 and qt == 3:
                    # split the last tile so the final store overlaps the copy
                    for c0 in (0, 256):
                        nc.scalar.activation(ot[:, c0:c0 + 256], pst[:, c0:c0 + 256], AF.Copy)
                        nc.sync.dma_start(
                            out_d.ap()[qt * 128:(qt + 1) * 128, nt * 512 + c0:nt * 512 + c0 + 256],
                            ot[:, c0:c0 + 256],
                        )
                else:
                    nc.scalar.activation(ot[:], pst[:, 0:512], AF.Copy)
                    nc.sync.dma_start(out_d.ap()[qt * 128:(qt + 1) * 128, nt * 512:(nt + 1) * 512], ot[:])

    nc.compile()
    return nc


def get_nc():
    if "nc" not in _CACHE:
        _CACHE["nc"] = _build_nc()
    return _CACHE["nc"]


def make_in_maps(x, wq, bq, wk, bk, wv, bv, wo):
    bf16 = ml_dtypes.bfloat16
    perm = np.concatenate([np.arange(0, HD, 2), np.arange(1, HD, 2)])
    qcols = np.concatenate([h * HD + perm for h in range(H)])
    kcols = np.concatenate([h * HD + perm for h in range(KVH)])
    wq_p = wq[:, qcols]
    bq_p = np.ascontiguousarray(bq[qcols].reshape(H, HD).T).astype(np.float32)
    wk_p = wk[:, kcols]
    bk_p = np.ascontiguousarray(bk[kcols].reshape(KVH, HD).T).astype(np.float32)
    # pretile so every DMA is contiguous: wq [ht][p][ks][c], wk/wv [p][ks][c],
    # wo [nt][p][ct][c]
    wq_t = np.ascontiguousarray(
        wq_p.reshape(KS, 128, H, 128).transpose(2, 1, 0, 3).reshape(H, 128, KS * 128)
    ).astype(bf16)
    wk_t = np.ascontiguousarray(
        wk_p.reshape(KS, 128, DKV).transpose(1, 0, 2).reshape(128, KS * DKV)
    ).astype(bf16)
    wv_t = np.ascontiguousarray(
        wv.reshape(KS, 128, DKV).transpose(1, 0, 2).reshape(128, KS * DKV)
    ).astype(bf16)
    wo_t = np.ascontiguousarray(
        wo.reshape(KS, 128, 4, 512).transpose(2, 1, 0, 3).reshape(4, 128, KS * 512)
    ).astype(bf16)
    bv_rep = np.tile(bv.astype(np.float32), (128, 1))
    theta = (10000.0 ** (-np.arange(64, dtype=np.float64) / 64.0))
    ang = np.outer(np.arange(S, dtype=np.float64), theta)  # [S, 64]
    c = np.cos(ang).T.astype(np.float32)  # [64, S]
    s = np.sin(ang).T.astype(np.float32)
    cosT = np.concatenate([c, c], axis=0)      # [128, S]
    sinT = np.concatenate([-s, s], axis=0)     # [128, S]
    ones = np.ones((128, 128), dtype=bf16)
    # psw = perm.T @ qf must be the half-swap: psw[i] = qf[(i+64)%128]
    perm = np.roll(np.eye(128, dtype=np.float32), 64, axis=0).astype(bf16)

    in_maps = []
    for b in range(B):
        for g in range(G):
            sl = slice(g * SQ, (g + 1) * SQ)
            xt_c = np.ascontiguousarray(
                x[b, sl, :].T.reshape(KS, 128, SQ).transpose(1, 0, 2).reshape(128, KS * SQ)
            ).astype(bf16)
            in_maps.append({
                "xt": xt_c,
                "wq": wq_t, "wk": wk_t, "wv": wv_t, "wo": wo_t,
                "bq": bq_p, "bk": bk_p, "bv": bv_rep,
                "cosq": np.ascontiguousarray(cosT[:, sl]).astype(bf16),
                "sinq": np.ascontiguousarray(sinT[:, sl]).astype(bf16),
                "ones": ones,
                "perm": perm,
            })
    return in_maps


def assemble(results):
    out = np.empty((B, S, D), np.float32)
    for b in range(B):
        for g in range(G):
            out[b, g * SQ:(g + 1) * SQ, :] = results[b * G + g]["out"]
    return out


def kernel(x, wq, bq, wk, bk, wv, bv, wo):
    from concourse.bass_utils import run_bass_kernel_spmd

    x, wq, bq, wk, bk, wv, bv, wo = (
        np.asarray(t, dtype=np.float32) for t in (x, wq, bq, wk, bk, wv, bv, wo)
    )
    nc = get_nc()
    in_maps = make_in_maps(x, wq, bq, wk, bk, wv, bv, wo)
    # run twice and return the second result: the first execution after a
    # NEFF load has occasionally produced stale collective output.
    run_bass_kernel_spmd(nc, in_maps, core_ids=list(range(NCORES)))
    res = run_bass_kernel_spmd(nc, in_maps, core_ids=list(range(NCORES)))
    return assemble(res.results)

